# revision 2
# baseline (speedup 1.0000x reference)
"""Trainium2 Bass kernel for a 16-head MHA layer (batch 4, seq 2048, embed 1024).

Sharding: 8 cores; core c handles batch c//2 and query-token half c%2.
Each core receives its batch's x rotated so that its 1024 query tokens sit in
rows 0:1024 (softmax/attention is permutation-invariant over key order).
K/V are computed over the full sequence on-core; no collectives. Weights are
replicated.

v2: fp8 DoubleRow matmuls. Host passes x pre-transposed and split into an
fp8(e4m3) hi/lo pair (x = x_hi + x_lo to ~bf16 accuracy), and Wqkv likewise.
Projections run as 3-term hi/lo DR chains (x_hi*W_hi + x_lo*W_hi + x_hi*W_lo);
scores run as DR with (K_hi, K_lo) stationary slots against a duplicated fp8 Q
(the only fp8 rounding that survives to the output, and the softmax damps it).
AV and the out-projection stay bf16. The K bias is dropped (softmax-invariant)
and the V bias + out bias are folded host-side into a single output bias row.
"""

import sys

for _p in ("/opt/trn_rl_repo",):
    if _p not in sys.path:
        sys.path.insert(0, _p)

import numpy as np
import ml_dtypes

import concourse.bass as bass  # noqa: E402
import concourse.mybir as mybir  # noqa: E402
import concourse.tile as tile  # noqa: E402
from concourse import bacc  # noqa: E402
from concourse.masks import make_identity  # noqa: E402

SEQ = 2048
E = 1024
H = 16
D = 64
NQ = 1024  # query tokens per core
N_CORES = 8

F32 = mybir.dt.float32
BF16 = mybir.dt.bfloat16
FP8 = mybir.dt.float8e4
FP8E5 = mybir.dt.float8e5
AF = mybir.ActivationFunctionType
DR = mybir.MatmulPerfMode.DoubleRow
ALU = mybir.AluOpType

E4M3 = ml_dtypes.float8_e4m3
E5M2 = ml_dtypes.float8_e5m2

ET = E // 128  # 8 e-chunks
EP = ET // 2  # 4 e-chunk pairs (DoubleRow)
TT = SEQ // 128  # 16 token tiles
QB = NQ // 512  # 2 query blocks
KT = SEQ // 128  # 16 key tiles
HP = H // 2  # 8 head pairs


def build_program():
    nc = bacc.Bacc(trn_type="TRN2", target_bir_lowering=False, debug=False)

    # x transposed: [e-chunk, 128 e-rows, token] fp8 hi/lo
    xth = nc.dram_tensor("xth", [ET, 128, SEQ], FP8, kind="ExternalInput").ap()
    xtl = nc.dram_tensor("xtl", [ET, 128, SEQ], FP8, kind="ExternalInput").ap()
    wqh = nc.dram_tensor("wqh", [E, 3 * E], FP8, kind="ExternalInput").ap()
    wql = nc.dram_tensor("wql", [E, 3 * E], FP8E5, kind="ExternalInput").ap()
    woh = nc.dram_tensor("woh", [E, E], FP8, kind="ExternalInput").ap()
    wol = nc.dram_tensor("wol", [E, E], FP8E5, kind="ExternalInput").ap()
    # Q bias transposed: column g holds bqkv[g*128:(g+1)*128] for g in 0..7
    bqt = nc.dram_tensor("bqt", [128, ET], F32, kind="ExternalInput").ap()
    # folded output bias row: bqkv_v @ Wo + bo
    bob = nc.dram_tensor("bob", [E], F32, kind="ExternalInput").ap()
    out = nc.dram_tensor("out", [NQ, E], BF16, kind="ExternalOutput").ap()

    with tile.TileContext(nc) as tc:
        _body(nc, tc, xth, xtl, wqh, wql, woh, wol, bqt, bob, out)

    nc.compile()
    return nc


def _body(nc, tc, xth, xtl, wqh, wql, woh, wol, bqt, bob, out):
    from contextlib import ExitStack

    es = ExitStack()
    with es:
        pc = es.enter_context(tc.tile_pool(name="const", bufs=1))
        pat = es.enter_context(tc.tile_pool(name="at", bufs=1))

        # --- constants -------------------------------------------------
        ident = pc.tile([128, 128], BF16, tag="ident")
        make_identity(nc, ident)
        bqtS = pc.tile([128, ET], F32, tag="bqtS")
        nc.sync.dma_start(out=bqtS, in_=bqt)
        boB = pc.tile([128, E], F32, tag="boB")
        bob_bcast = bass.AP(
            tensor=bob.tensor, offset=bob.offset, ap=[[0, 128]] + bob.ap
        )
        nc.gpsimd.dma_start(out=boB, in_=bob_bcast)

        # attn output (transposed), fp8 hi/lo pair for the DR out-proj;
        # chunk p holds heads 2p (rows 0:64) and 2p+1 (rows 64:128)
        ATh = pat.tile([128, HP, NQ], FP8, tag="ath")
        ATl = pat.tile([128, HP, NQ], FP8E5, tag="atl")

        with (
            tc.tile_pool(name="kqv", bufs=1) as pkqv,
            tc.tile_pool(name="xT", bufs=1) as pxt,
            tc.tile_pool(name="wpan", bufs=4) as pw,
            tc.tile_pool(name="wo", bufs=1) as pwo,
            tc.tile_pool(name="ph3", bufs=3) as p3,
        ):
            # K^T in fp8 hi/lo slots: KT2[hp][:, 0, :] = hi, [:, 1, :] = lo
            KT2 = [
                pkqv.tile([128, 2, SEQ], FP8, tag=f"kt{i}", name=f"ktt{i}")
                for i in range(ET)
            ]
            QT8 = [
                pkqv.tile([128, NQ], FP8, tag=f"qt{i}", name=f"qtt{i}")
                for i in range(ET)
            ]
            # V in AV-stationary layout: per key-tile, 8 head pairs of
            # [V_h0 | ones | V_h1] (64+64+64 cols); the shared ones column
            # block makes the denominator come out of the same matmul.
            VO = [
                pkqv.tile([128, HP, 192], BF16, tag=f"vo{i}", name=f"vo{i}")
                for i in range(TT)
            ]
            xh = pxt.tile([128, ET, SEQ], FP8, tag="xh")
            xl = pxt.tile([128, ET, SEQ], FP8, tag="xl")

            def dma_x_part(tb, which):
                # one hi/lo token-block slice (4KB/partition — big enough
                # that the 625ns HWDGE trigger hides under the transfer)
                t, src = (xh, xth) if which == 0 else (xl, xtl)
                nc.sync.dma_start(
                    out=t[:, :, tb * 512 : (tb + 1) * 512],
                    in_=bass.AP(
                        tensor=src.tensor,
                        offset=src.offset + tb * 512,
                        ap=[[SEQ, 128], [128 * SEQ, ET], [1, 512]],
                    ),
                )

            def dma_x(tb):
                dma_x_part(tb, 0)
                dma_x_part(tb, 1)

            def alloc_panel(pc0):
                return [
                    pw.tile(
                        [128, ET, 512],
                        FP8 if idx == 0 else FP8E5,
                        tag=f"wp{idx}",
                        name=f"wp{idx}_{pc0}",
                    )
                    for idx in range(2)
                ]

            def dma_panel(tiles, pc0, which):
                src = (wqh, wql)[which]
                nc.sync.dma_start(
                    out=tiles[which],
                    in_=bass.AP(
                        tensor=src.tensor,
                        offset=src.offset + pc0,
                        ap=[[3 * E, 128], [3 * E * 128, ET], [1, 512]],
                    ),
                )

            def load_panel(pc0):
                tiles = alloc_panel(pc0)
                dma_panel(tiles, pc0, 0)
                dma_panel(tiles, pc0, 1)
                return tiles

            def load_wo_panel(pc0):
                tiles = []
                for idx, src_w in enumerate((woh, wol)):
                    wp = pwo.tile(
                        [128, ET, 512],
                        FP8 if idx == 0 else FP8E5,
                        tag=f"wo{idx}_{pc0}",
                        name=f"wo{idx}_{pc0}",
                    )
                    nc.sync.dma_start(
                        out=wp,
                        in_=bass.AP(
                            tensor=src_w.tensor,
                            offset=src_w.offset + pc0,
                            ap=[[E, 128], [E * 128, ET], [1, 512]],
                        ),
                    )
                    tiles.append(wp)
                return tiles

            def dup2(ap_base):
                # [P, N] -> [P, 2, N] with stride-0 middle dim (shared slot)
                return bass.AP(
                    tensor=ap_base.tensor,
                    offset=ap_base.offset,
                    ap=[ap_base.ap[0], [0, 2], ap_base.ap[1]],
                )

            with tc.tile_pool(name="ps_proj", bufs=1, space="PSUM") as ppj:
                from contextlib import ExitStack as _ES2

                _att_es = _ES2()

                def dr_chain(ps, lhs_of, rhs_of, nterm=3):
                    # 3-term hi/lo fp8 chain: hi*hi, lo*hi, hi*lo over 4
                    # e-chunk pairs each; all accumulate into one PSUM tile.
                    terms = [(0, 0), (1, 0), (0, 1)][:nterm]
                    n = len(terms) * EP
                    k = 0
                    for tl, tr in terms:
                        for j in range(EP):
                            nc.tensor.matmul(
                                ps,
                                lhsT=lhs_of(tl, j),
                                rhs=rhs_of(tr, j),
                                start=(k == 0),
                                stop=(k == n - 1),
                                perf_mode=DR,
                            )
                            k += 1

                def v_chain(wp, panel, tt):
                    # one V-proj output tile -> VO pair layout (+ ones memset)
                    wph, wpl = wp
                    p0 = panel * 4
                    ps = ppj.tile([128, 512], F32, tag="ps", bufs=2)
                    xx = (xh, xl)
                    ww = (wph, wpl)
                    dr_chain(
                        ps,
                        lambda tl, j: xx[tl][:, 2 * j : 2 * j + 2, tt * 128 : (tt + 1) * 128],
                        lambda tr, j: ww[tr][:, 2 * j : 2 * j + 2, :],
                    )
                    ps3 = ps.rearrange("p (pr d) -> p pr d", d=128)
                    nc.vector.tensor_copy(VO[tt][:, p0 : p0 + 4, 0:64], ps3[:, :, 0:64])
                    nc.vector.tensor_copy(
                        VO[tt][:, p0 : p0 + 4, 128:192], ps3[:, :, 64:128]
                    )
                    if panel == 0:
                        nc.vector.memset(VO[tt][:, :, 64:128], 1.0)

                def kq_chain(wp, kind, panel, ct, tb):
                    # one K^T/Q^T-proj output tile (Q gets bias; K needs none —
                    # a per-query additive constant is softmax-invariant)
                    wph, wpl = wp
                    gct = panel * 4 + ct
                    ps = ppj.tile([128, 512], F32, tag="ps", bufs=2)
                    xx = (xh, xl)
                    ww = (wph, wpl)
                    dr_chain(
                        ps,
                        lambda tl, j: ww[tl][:, 2 * j : 2 * j + 2, ct * 128 : (ct + 1) * 128],
                        lambda tr, j: xx[tr][:, 2 * j : 2 * j + 2, tb * 512 : (tb + 1) * 512],
                        # lhs/rhs term roles are swapped relative to v_chain:
                        # terms needed are (Wh,xh), (Wh,xl), (Wl,xh)
                    )
                    tsl = slice(tb * 512, (tb + 1) * 512)
                    if kind == "q":
                        nc.vector.tensor_scalar_add(
                            QT8[gct][:, tsl], ps, bqtS[:, gct : gct + 1]
                        )
                    else:
                        nc.vector.tensor_copy(KT2[gct][:, 0, tsl], ps)
                        nc.vector.scalar_tensor_tensor(
                            KT2[gct][:, 1, tsl],
                            ps,
                            0.0,
                            KT2[gct][:, 0, tsl],
                            ALU.add,
                            ALU.subtract,
                        )

                def make_block(hp, q0, qw):
                    # Flat cross-block software pipeline: the caller emits
                    # scores/exp for (b, kt) and the AV of the PREVIOUS
                    # (b, kt) pair, so the exp stream never queues behind a
                    # block boundary. qw is the query width (256 for the
                    # split final block).
                    av = [
                        pav.tile([128, 512], F32, tag="av", name=f"av{hp}_{q0}_{i}")
                        for i in range(2)
                    ]
                    pts = [None] * KT
                    blk = {}

                    def scores(kt):
                        k0 = kt * 128
                        ps_s = pss.tile([128, 1024], F32, tag="ps_s")
                        for i in range(2):
                            r0 = i * 64
                            nc.tensor.matmul(
                                ps_s[:, i * qw : (i + 1) * qw],
                                lhsT=KT2[hp][r0 : r0 + 64, :, k0 : k0 + 128],
                                rhs=dup2(QT8[hp][r0 : r0 + 64, q0 : q0 + qw]),
                                start=True,
                                stop=True,
                                perf_mode=DR,
                            )
                        pt = p3.tile([128, 1024], BF16, tag="pt", bufs=3)
                        nc.scalar.activation(
                            pt[:, 0 : 2 * qw], ps_s[:, 0 : 2 * qw], AF.Exp, scale=0.125
                        )
                        pts[kt] = pt

                    def emit_av(kt):
                        for i in range(2):
                            # i=0: rows 0:64 = V.T @ P, rows 64:128 = denom
                            # i=1: rows 0:64 = denom, rows 64:128 = V.T @ P
                            nc.tensor.matmul(
                                av[i][:, 0:qw],
                                lhsT=VO[kt][:, hp, 64 * i : 64 * i + 128],
                                rhs=pts[kt][:, i * qw : (i + 1) * qw],
                                start=(kt == 0),
                                stop=(kt == KT - 1),
                            )
                        pts[kt] = None

                    avsbs = []
                    atf = [None]

                    def realign_copy():
                        # bf16 copies of both AV psums, emitted right after the
                        # final AV so the DVE is done before the PE selects
                        # reach the queue head (a PE-queue stall otherwise).
                        for i in range(2):
                            av_sb = p3.tile([128, 512], BF16, tag="avsb", bufs=2)
                            nc.vector.tensor_copy(av_sb[:, 0:qw], av[i][:, 0:qw])
                            avsbs.append(av_sb)

                    def realign():
                        # DVE ops need all inputs at base partition 0; realign
                        # the half at rows 64:128 via a PE select-matmul.
                        for i in range(2):
                            rec = p3.tile([64, 512], F32, tag="rec", bufs=2)
                            av_sb = avsbs[i]
                            # selects share the double-buffered chain-psum ring
                            sel_t = ppj.tile([128, 512], F32, tag="ps", bufs=2)
                            sel = sel_t[0:64, 0:qw]
                            nc.tensor.matmul(
                                sel, lhsT=ident[:, 64:128], rhs=av_sb[:, 0:qw],
                                start=True, stop=True,
                            )
                            if i == 0:
                                atf[0] = p3.tile([128, 512], BF16, tag="atf", bufs=2, name=f"atf{hp}_{q0}")
                                nc.vector.reciprocal_approx_fast(rec[:, 0:qw], sel)
                                nc.vector.tensor_mul(
                                    atf[0][0:64, 0:qw], av[i][0:64, 0:qw], rec[:, 0:qw]
                                )
                            else:
                                nc.vector.reciprocal_approx_fast(
                                    rec[:, 0:qw], av[i][0:64, 0:qw]
                                )
                                # out at base partition 64 with inputs at 0 is
                                # fine for TensorTensor (not for stt below)
                                nc.vector.tensor_mul(
                                    atf[0][64:128, 0:qw], sel, rec[:, 0:qw]
                                )

                    def at_split():
                        # per half so every AP in the stt shares its start
                        # partition (verifier requirement)
                        for i in range(2):
                            r0 = i * 64
                            hi = ATh[r0 : r0 + 64, hp, q0 : q0 + qw]
                            src_h = atf[0][r0 : r0 + 64, 0:qw]
                            nc.vector.tensor_copy(hi, src_h)
                            nc.vector.scalar_tensor_tensor(
                                ATl[r0 : r0 + 64, hp, q0 : q0 + qw],
                                src_h,
                                0.0,
                                hi,
                                ALU.add,
                                ALU.subtract,
                            )

                    blk["scores"] = scores
                    blk["av"] = emit_av
                    blk["realign_copy"] = realign_copy
                    blk["realign"] = realign
                    blk["at_split"] = at_split
                    return blk

                def outproj_chain(tt, half, wo_p):
                    c0 = half * 512
                    ps = ppj.tile([128, 512], F32, tag="ps", bufs=2)
                    tsl = slice(tt * 128, (tt + 1) * 128)
                    aa = (ATh, ATl)
                    ww = wo_p
                    k = 0
                    for tl, tr in ((0, 0), (1, 0), (0, 1)):
                        for j in range(EP):
                            nc.tensor.matmul(
                                ps,
                                lhsT=aa[tl][:, 2 * j : 2 * j + 2, tsl],
                                rhs=ww[tr][:, 2 * j : 2 * j + 2, :],
                                start=(k == 0),
                                stop=(k == 3 * EP - 1),
                                perf_mode=DR,
                            )
                            k += 1
                    osb = p3.tile([128, 512], BF16, tag="osb", bufs=2)
                    nc.vector.tensor_add(osb, ps, boB[:, c0 : c0 + 512])
                    nc.sync.dma_start(
                        out=out[tt * 128 : (tt + 1) * 128, c0 : c0 + 512], in_=osb
                    )

                # --- prologue. DMA device is serial, so emission order is
                # arrival order; order strictly by first need, with chains
                # emitted as soon as their data is in flight. Block 0 starts
                # after just k0 tb0/tb1, six v0 chains, and q0 tb0; the
                # remaining block-0 prerequisites stream in as its inner work.
                wp_k0 = alloc_panel(E)
                dma_panel(wp_k0, E, 0)
                dma_panel(wp_k0, E, 1)
                dma_x(0)
                wp_q0 = alloc_panel(0)
                dma_panel(wp_q0, 0, 0)
                dma_panel(wp_q0, 0, 1)
                wp_v0 = load_panel(2 * E)
                dma_x(1)
                dma_x(2)
                dma_x(3)
                kq_chain(wp_k0, "k", 0, 0, 0)
                kq_chain(wp_q0, "q", 0, 0, 0)

                pss = _att_es.enter_context(
                    tc.tile_pool(name="ps_s", bufs=2, space="PSUM")
                )
                pav = _att_es.enter_context(
                    tc.tile_pool(name="ps_av", bufs=2, space="PSUM")
                )
                # --- chain schedule. V panel 0 feeds block 0 just-in-time
                # (v_chain(kt) emitted at iteration kt); V panel 1 likewise
                # splits across blocks 7/8. K/Q coltile ct of panel p feeds
                # head pair hp = 4p+ct, consumed from block 2hp on — each
                # chain group is emitted in an earlier block.
                wp_v1 = [None]
                wp_k1 = [None]
                wp_q1 = [None]
                wo_panels = [None, None]

                def kq_group(kind, panel, ct):
                    wp = {("k", 0): wp_k0, ("q", 0): wp_q0}.get((kind, panel))
                    n = 4 if kind == "k" else 2
                    if wp is not None:
                        return [
                            (lambda t=tb: kq_chain(wp, kind, panel, ct, t))
                            for tb in range(n)
                        ]
                    box = wp_k1 if kind == "k" else wp_q1
                    return [
                        (lambda t=tb: kq_chain(box[0], kind, panel, ct, t))
                        for tb in range(n)
                    ]

                def spread(chs):
                    # spread chain closures over kt slots 1..14
                    n = len(chs)
                    slots = [1 + (i * 14) // n for i in range(n)]
                    inner = {}
                    for s, ch in zip(slots, chs):
                        inner.setdefault(s, []).append(ch)
                    return inner

                def v1_group(tts):
                    return [(lambda t=tt: v_chain(wp_v1[0], 1, t)) for tt in tts]

                # per-block inner chain assignments (block -> kt -> chains),
                # balanced so no block carries more than ~4 chains beyond the
                # attention work, with every group emitted before its
                # consumer block (K/Q ct of panel p -> block 2*(4p+ct);
                # V panel 1 -> block 8).
                k0g = {c: kq_group("k", 0, c) for c in (1, 2, 3)}
                q0g = {c: kq_group("q", 0, c) for c in (1, 2, 3)}
                k1g = {c: kq_group("k", 1, c) for c in (0, 1, 2, 3)}
                q1g = {c: kq_group("q", 1, c) for c in (0, 1, 2, 3)}
                b0_inner = {
                    tt + 1: [lambda t=tt: v_chain(wp_v0, 0, t)] for tt in range(15)
                }
                b0_inner[15].append(lambda: v_chain(wp_v0, 0, 15))
                b0_inner[3] = b0_inner.get(3, []) + [
                    lambda: kq_chain(wp_k0, "k", 0, 0, 1)
                ]
                b0_inner[6].append(lambda: kq_chain(wp_k0, "k", 0, 0, 2))
                b0_inner[10].append(lambda: kq_chain(wp_k0, "k", 0, 0, 3))
                b0_inner[12].append(lambda: kq_chain(wp_q0, "q", 0, 0, 1))
                binner = {
                    0: b0_inner,
                    1: spread(k0g[1] + q0g[1]),
                    2: spread(k0g[2]),
                    3: spread(q0g[2] + v1_group(range(0, 2))),
                    4: spread(k0g[3]),
                    5: spread(q0g[3] + v1_group(range(2, 4))),
                    6: spread(k1g[0] + q1g[0]),
                    7: spread(v1_group(range(4, 10))),
                    8: {tt: [ch] for tt, ch in zip(range(10, 16), v1_group(range(10, 16)))},
                    9: spread(k1g[1] + q1g[1][:1]),
                    10: spread(q1g[1][1:] + k1g[2]),
                    11: spread(q1g[2] + k1g[3][:2]),
                    12: spread(k1g[3][2:] + q1g[3][:1]),
                    13: spread(q1g[3][1:]),
                    15: {
                        4 + i: [
                            (lambda t=tt, h=half: outproj_chain(t, h, wo_panels[h]))
                        ]
                        for i, (tt, half) in enumerate(
                            (t, h) for t in range(4) for h in range(2)
                        )
                    },
                }

                blocks = [(hp, qb * 512, 512) for hp in range(HP) for qb in range(QB)]
                pending = None
                prev = None  # (blk, kt) still owing its AV
                for b, (hp, q0, qw) in enumerate(blocks):
                    if b == 1:
                        # 4-slot wp ring: v1 lands in a fresh slot
                        wp_v1[0] = load_panel(2 * E + 512)
                    if b == 6:
                        # k1/q1 reuse k0/q0 slots; every k0/q0 reader has
                        # been emitted by now, so the anti-deps resolve.
                        wp_k1[0] = load_panel(E + 512)
                        wp_q1[0] = load_panel(512)
                    if b == 12:
                        wo_panels[0] = load_wo_panel(0)
                        wo_panels[1] = load_wo_panel(512)
                    blk = make_block(hp, q0, qw)
                    inner = binner.get(b, {})
                    for kt in range(KT):
                        blk["scores"](kt)
                        if kt == 1 and pending is not None:
                            pending[1]()
                        if kt == 3 and pending is not None:
                            pending[2]()
                        for ch in inner.get(kt, ()):
                            ch()
                        if prev is not None:
                            prev[0]["av"](prev[1])
                            if kt == 0 and pending is not None:
                                pending[0]()
                        prev = (blk, kt)
                    pending = (blk["realign_copy"], blk["realign"], blk["at_split"])
                prev[0]["av"](prev[1])
                pending[0]()
                pending[1]()
                pending[2]()
                for tt in range(4, 8):
                    for half in range(2):
                        outproj_chain(tt, half, wo_panels[half])
                _att_es.close()


_NC = None


def _get_program():
    global _NC
    if _NC is None:
        _NC = build_program()
    return _NC


def _split8(a):
    hi = a.astype(E4M3)
    lo = (a - hi.astype(np.float32)).astype(E4M3)
    return hi, lo


def make_in_maps(x, Wqkv, bqkv, Wo, bo):
    Wqkv = np.asarray(Wqkv, np.float32)
    bqkv = np.asarray(bqkv, np.float32)
    Wo = np.asarray(Wo, np.float32)
    bo = np.asarray(bo, np.float32)
    x = np.asarray(x, np.float32)

    wqh = Wqkv.astype(E4M3)
    wql = (Wqkv - wqh.astype(np.float32)).astype(E5M2)
    woh = Wo.astype(E4M3)
    wol = (Wo - woh.astype(np.float32)).astype(E5M2)
    bqt = np.ascontiguousarray(bqkv[:E].reshape(ET, 128).T)  # [128, 8]
    bob = bqkv[2 * E :] @ Wo + bo  # folded V-bias + out-bias
    w = {
        "wqh": np.ascontiguousarray(wqh),
        "wql": np.ascontiguousarray(wql),
        "woh": np.ascontiguousarray(woh),
        "wol": np.ascontiguousarray(wol),
        "bqt": bqt.astype(np.float32),
        "bob": bob.astype(np.float32),
    }
    in_maps = []
    for c in range(N_CORES):
        b, s = divmod(c, 2)
        xb = x[b]
        if s == 1:
            xb = np.roll(xb, -NQ, axis=0)
        xT = np.ascontiguousarray(xb.T).reshape(ET, 128, SEQ)
        xh, xl = _split8(xT)
        in_maps.append(
            {"xth": np.ascontiguousarray(xh), "xtl": np.ascontiguousarray(xl), **w}
        )
    return in_maps


def gather_out(results):
    out = np.empty((4, SEQ, E), np.float32)
    for c in range(N_CORES):
        b, s = divmod(c, 2)
        out[b, s * NQ : (s + 1) * NQ] = results[c]["out"].astype(np.float32)
    return out


def kernel(x, Wqkv, bqkv, Wo, bo):
    from concourse.bass_utils import run_bass_kernel_spmd

    nc = _get_program()
    in_maps = make_in_maps(x, Wqkv, bqkv, Wo, bo)
    res = run_bass_kernel_spmd(nc, in_maps, core_ids=list(range(N_CORES)))
    return gather_out(res.results)


# revision 5
# speedup vs baseline: 1.0087x; 1.0087x over previous
"""Trainium2 Bass kernel for a 16-head MHA layer (batch 4, seq 2048, embed 1024).

Sharding: 8 cores; core c handles batch c//2 and query-token half c%2.
Each core receives its batch's x rotated so its 1024 query tokens sit in rows
0:1024 (attention is permutation-invariant over key order). K/V are computed
over the full sequence on-core; weights are replicated; no collectives.

Numerics/compute strategy (fp8 DoubleRow on the PE at 0.5 cycles/row):
- Host passes x pre-transposed and split hi/lo: x = e4m3(x) + e4m3(residual),
  and Wqkv/Wo split as e4m3(W) + e5m2(residual) (the residual of the tiny
  uniform weights underflows e4m3's subnormals, hence e5m2).
- QKV projections: 3-term hi/lo DR chains (xh*Wh + xl*Wh + xh*Wl), e-chunk
  pairs in the two DR slots.
- Scores: DR with (K_hi, K_lo) stationary slots against a stride-0-duplicated
  e4m3 Q. Only the Q rounding survives to the output and the softmax damps it.
- The softmax denominator comes free from ones-columns in the AV stationary
  tile; exp runs on ACT straight out of PSUM; AV stays bf16.
- Out-projection: 3-term DR over an e4m3/e5m2 split of the attention output.
- The K bias is dropped (additive per-query constants are softmax-invariant);
  the V bias and output bias fold host-side into one bias row.

Schedule: a flat software pipeline over (block, kt) — AV lags scores/exp by
one step and crosses block boundaries, the realign/AT-split DVE work of each
block is deferred into the next block, and projection chains are packed into
the back half (kts 12..15) of earlier blocks so the exp stream stays hot.
"""

import sys

for _p in ("/opt/trn_rl_repo",):
    if _p not in sys.path:
        sys.path.insert(0, _p)

import numpy as np
import ml_dtypes

import concourse.bass as bass  # noqa: E402
import concourse.mybir as mybir  # noqa: E402
import concourse.tile as tile  # noqa: E402
from concourse import bacc  # noqa: E402
from concourse.masks import make_identity  # noqa: E402

SEQ = 2048
E = 1024
H = 16
D = 64
NQ = 1024  # query tokens per core
N_CORES = 8

F32 = mybir.dt.float32
BF16 = mybir.dt.bfloat16
FP8 = mybir.dt.float8e4
FP8E5 = mybir.dt.float8e5
AF = mybir.ActivationFunctionType
DR = mybir.MatmulPerfMode.DoubleRow
ALU = mybir.AluOpType

E4M3 = ml_dtypes.float8_e4m3
E5M2 = ml_dtypes.float8_e5m2

ET = E // 128  # 8 e-chunks
EP = ET // 2  # 4 e-chunk pairs (DoubleRow)
TT = SEQ // 128  # 16 token tiles
QB = NQ // 512  # 2 query blocks
KT = SEQ // 128  # 16 key tiles
HP = H // 2  # 8 head pairs


def build_program():
    nc = bacc.Bacc(trn_type="TRN2", target_bir_lowering=False, debug=False)

    # x transposed: [e-chunk, 128 e-rows, token] fp8 hi/lo
    xth = nc.dram_tensor("xth", [ET, 128, SEQ], FP8, kind="ExternalInput").ap()
    xtl = nc.dram_tensor("xtl", [ET, 128, SEQ], FP8, kind="ExternalInput").ap()
    wqh = nc.dram_tensor("wqh", [E, 3 * E], FP8, kind="ExternalInput").ap()
    wql = nc.dram_tensor("wql", [E, 3 * E], FP8E5, kind="ExternalInput").ap()
    woh = nc.dram_tensor("woh", [E, E], FP8, kind="ExternalInput").ap()
    wol = nc.dram_tensor("wol", [E, E], FP8E5, kind="ExternalInput").ap()
    # Q bias transposed: column g holds bqkv[g*128:(g+1)*128] for g in 0..7
    bqt = nc.dram_tensor("bqt", [128, ET], F32, kind="ExternalInput").ap()
    # folded output bias row: bqkv_v @ Wo + bo
    bob = nc.dram_tensor("bob", [E], F32, kind="ExternalInput").ap()
    out = nc.dram_tensor("out", [NQ, E], BF16, kind="ExternalOutput").ap()

    with tile.TileContext(nc) as tc:
        _body(nc, tc, xth, xtl, wqh, wql, woh, wol, bqt, bob, out)

    nc.compile()
    return nc


def _body(nc, tc, xth, xtl, wqh, wql, woh, wol, bqt, bob, out):
    from contextlib import ExitStack

    es = ExitStack()
    with es:
        pc = es.enter_context(tc.tile_pool(name="const", bufs=1))
        pat = es.enter_context(tc.tile_pool(name="at", bufs=1))

        # --- constants -------------------------------------------------
        ident = pc.tile([128, 128], BF16, tag="ident")
        make_identity(nc, ident)
        bqtS = pc.tile([128, ET], F32, tag="bqtS")
        nc.sync.dma_start(out=bqtS, in_=bqt)
        boB = pc.tile([128, E], F32, tag="boB")
        bob_bcast = bass.AP(
            tensor=bob.tensor, offset=bob.offset, ap=[[0, 128]] + bob.ap
        )
        nc.gpsimd.dma_start(out=boB, in_=bob_bcast)

        # attn output (transposed), fp8 hi/lo pair for the DR out-proj;
        # chunk p holds heads 2p (rows 0:64) and 2p+1 (rows 64:128)
        ATh = pat.tile([128, HP, NQ], FP8, tag="ath")
        ATl = pat.tile([128, HP, NQ], FP8E5, tag="atl")

        with (
            tc.tile_pool(name="kqv", bufs=1) as pkqv,
            tc.tile_pool(name="xT", bufs=1) as pxt,
            tc.tile_pool(name="wpan", bufs=4) as pw,
            tc.tile_pool(name="wo", bufs=1) as pwo,
            tc.tile_pool(name="ph3", bufs=3) as p3,
        ):
            # K^T in fp8 hi/lo slots: KT2[hp][:, 0, :] = hi, [:, 1, :] = lo
            KT2 = [
                pkqv.tile([128, 2, SEQ], FP8, tag=f"kt{i}", name=f"ktt{i}")
                for i in range(ET)
            ]
            QT8 = [
                pkqv.tile([128, NQ], FP8, tag=f"qt{i}", name=f"qtt{i}")
                for i in range(ET)
            ]
            # V in AV-stationary layout: per key-tile, 8 head pairs of
            # [V_h0 | ones | V_h1] (64+64+64 cols); the shared ones column
            # block makes the denominator come out of the same matmul.
            VO = [
                pkqv.tile([128, HP, 192], BF16, tag=f"vo{i}", name=f"vo{i}")
                for i in range(TT)
            ]
            xh = pxt.tile([128, ET, SEQ], FP8, tag="xh")
            xl = pxt.tile([128, ET, SEQ], FP8, tag="xl")

            def dma_x_part(tb, which):
                # one hi/lo token-block slice (4KB/partition — big enough
                # that the 625ns HWDGE trigger hides under the transfer)
                t, src = (xh, xth) if which == 0 else (xl, xtl)
                nc.sync.dma_start(
                    out=t[:, :, tb * 512 : (tb + 1) * 512],
                    in_=bass.AP(
                        tensor=src.tensor,
                        offset=src.offset + tb * 512,
                        ap=[[SEQ, 128], [128 * SEQ, ET], [1, 512]],
                    ),
                )

            def dma_x(tb):
                dma_x_part(tb, 0)
                dma_x_part(tb, 1)

            def alloc_panel(pc0):
                return [
                    pw.tile(
                        [128, ET, 512],
                        FP8 if idx == 0 else FP8E5,
                        tag=f"wp{idx}",
                        name=f"wp{idx}_{pc0}",
                    )
                    for idx in range(2)
                ]

            def dma_panel(tiles, pc0, which, c0=0, cn=512):
                src = (wqh, wql)[which]
                nc.sync.dma_start(
                    out=tiles[which][:, :, c0 : c0 + cn],
                    in_=bass.AP(
                        tensor=src.tensor,
                        offset=src.offset + pc0 + c0,
                        ap=[[3 * E, 128], [3 * E * 128, ET], [1, cn]],
                    ),
                )

            def load_panel(pc0):
                tiles = alloc_panel(pc0)
                dma_panel(tiles, pc0, 0)
                dma_panel(tiles, pc0, 1)
                return tiles

            def load_wo_panel(pc0):
                tiles = []
                for idx, src_w in enumerate((woh, wol)):
                    wp = pwo.tile(
                        [128, ET, 512],
                        FP8 if idx == 0 else FP8E5,
                        tag=f"wo{idx}_{pc0}",
                        name=f"wo{idx}_{pc0}",
                    )
                    nc.sync.dma_start(
                        out=wp,
                        in_=bass.AP(
                            tensor=src_w.tensor,
                            offset=src_w.offset + pc0,
                            ap=[[E, 128], [E * 128, ET], [1, 512]],
                        ),
                    )
                    tiles.append(wp)
                return tiles

            def dup2(ap_base):
                # [P, N] -> [P, 2, N] with stride-0 middle dim (shared slot)
                return bass.AP(
                    tensor=ap_base.tensor,
                    offset=ap_base.offset,
                    ap=[ap_base.ap[0], [0, 2], ap_base.ap[1]],
                )

            with tc.tile_pool(name="ps_proj", bufs=1, space="PSUM") as ppj:
                from contextlib import ExitStack as _ES2

                _att_es = _ES2()

                def dr_chain(ps, lhs_of, rhs_of, nterm=3):
                    # 3-term hi/lo fp8 chain: hi*hi, lo*hi, hi*lo over 4
                    # e-chunk pairs each; all accumulate into one PSUM tile.
                    terms = [(0, 0), (1, 0), (0, 1)][:nterm]
                    n = len(terms) * EP
                    k = 0
                    for tl, tr in terms:
                        for j in range(EP):
                            nc.tensor.matmul(
                                ps,
                                lhsT=lhs_of(tl, j),
                                rhs=rhs_of(tr, j),
                                start=(k == 0),
                                stop=(k == n - 1),
                                perf_mode=DR,
                            )
                            k += 1

                def v_chain(wp, panel, tt):
                    # one V-proj output tile -> VO pair layout (+ ones memset)
                    wph, wpl = wp
                    p0 = panel * 4
                    ps = ppj.tile([128, 512], F32, tag="ps", bufs=2)
                    xx = (xh, xl)
                    ww = (wph, wpl)
                    dr_chain(
                        ps,
                        lambda tl, j: xx[tl][:, 2 * j : 2 * j + 2, tt * 128 : (tt + 1) * 128],
                        lambda tr, j: ww[tr][:, 2 * j : 2 * j + 2, :],
                    )
                    ps3 = ps.rearrange("p (pr d) -> p pr d", d=128)
                    nc.vector.tensor_copy(VO[tt][:, p0 : p0 + 4, 0:64], ps3[:, :, 0:64])
                    nc.vector.tensor_copy(
                        VO[tt][:, p0 : p0 + 4, 128:192], ps3[:, :, 64:128]
                    )
                    if panel == 0:
                        nc.vector.memset(VO[tt][:, :, 64:128], 1.0)

                def kq_chain(wp, kind, panel, ct, tb):
                    # one K^T/Q^T-proj output tile (Q gets bias; K needs none —
                    # a per-query additive constant is softmax-invariant)
                    wph, wpl = wp
                    gct = panel * 4 + ct
                    ps = ppj.tile([128, 512], F32, tag="ps", bufs=2)
                    xx = (xh, xl)
                    ww = (wph, wpl)
                    dr_chain(
                        ps,
                        lambda tl, j: ww[tl][:, 2 * j : 2 * j + 2, ct * 128 : (ct + 1) * 128],
                        lambda tr, j: xx[tr][:, 2 * j : 2 * j + 2, tb * 512 : (tb + 1) * 512],
                        # lhs/rhs term roles are swapped relative to v_chain:
                        # terms needed are (Wh,xh), (Wh,xl), (Wl,xh)
                    )
                    tsl = slice(tb * 512, (tb + 1) * 512)
                    if kind == "q":
                        nc.vector.tensor_scalar_add(
                            QT8[gct][:, tsl], ps, bqtS[:, gct : gct + 1]
                        )
                    else:
                        nc.vector.tensor_copy(KT2[gct][:, 0, tsl], ps)
                        nc.vector.scalar_tensor_tensor(
                            KT2[gct][:, 1, tsl],
                            ps,
                            0.0,
                            KT2[gct][:, 0, tsl],
                            ALU.add,
                            ALU.subtract,
                        )

                def make_block(hp, q0, qw):
                    # Flat cross-block software pipeline: the caller emits
                    # scores/exp for (b, kt) and the AV of the PREVIOUS
                    # (b, kt) pair, so the exp stream never queues behind a
                    # block boundary. qw is the query width (256 for the
                    # split final block).
                    av = [
                        pav.tile([128, 512], F32, tag="av", name=f"av{hp}_{q0}_{i}")
                        for i in range(2)
                    ]
                    pts = [None] * KT
                    blk = {}

                    def scores(kt):
                        k0 = kt * 128
                        ps_s = pss.tile([128, 1024], F32, tag="ps_s")
                        for i in range(2):
                            r0 = i * 64
                            nc.tensor.matmul(
                                ps_s[:, i * qw : (i + 1) * qw],
                                lhsT=KT2[hp][r0 : r0 + 64, :, k0 : k0 + 128],
                                rhs=dup2(QT8[hp][r0 : r0 + 64, q0 : q0 + qw]),
                                start=True,
                                stop=True,
                                perf_mode=DR,
                            )
                        pt = p3.tile([128, 1024], BF16, tag="pt", bufs=4)
                        nc.scalar.activation(
                            pt[:, 0 : 2 * qw], ps_s[:, 0 : 2 * qw], AF.Exp, scale=0.125
                        )
                        pts[kt] = pt

                    def emit_av(kt):
                        for i in range(2):
                            # i=0: rows 0:64 = V.T @ P, rows 64:128 = denom
                            # i=1: rows 0:64 = denom, rows 64:128 = V.T @ P
                            nc.tensor.matmul(
                                av[i][:, 0:qw],
                                lhsT=VO[kt][:, hp, 64 * i : 64 * i + 128],
                                rhs=pts[kt][:, i * qw : (i + 1) * qw],
                                start=(kt == 0),
                                stop=(kt == KT - 1),
                            )
                        pts[kt] = None

                    avsbs = []
                    atf = [None]

                    def realign_copy():
                        # bf16 copies of both AV psums, emitted right after the
                        # final AV so the DVE is done before the PE selects
                        # reach the queue head (a PE-queue stall otherwise).
                        for i in range(2):
                            av_sb = p3.tile([128, 512], BF16, tag="avsb", bufs=2)
                            nc.vector.tensor_copy(av_sb[:, 0:qw], av[i][:, 0:qw])
                            avsbs.append(av_sb)

                    def realign():
                        # DVE ops need all inputs at base partition 0; realign
                        # the half at rows 64:128 via a PE select-matmul.
                        for i in range(2):
                            rec = p3.tile([64, 512], F32, tag="rec", bufs=2)
                            av_sb = avsbs[i]
                            # selects share the double-buffered chain-psum ring
                            sel_t = ppj.tile([128, 512], F32, tag="ps", bufs=2)
                            sel = sel_t[0:64, 0:qw]
                            nc.tensor.matmul(
                                sel, lhsT=ident[:, 64:128], rhs=av_sb[:, 0:qw],
                                start=True, stop=True,
                            )
                            if i == 0:
                                atf[0] = p3.tile([128, 512], BF16, tag="atf", bufs=2, name=f"atf{hp}_{q0}")
                                nc.vector.reciprocal_approx_fast(rec[:, 0:qw], sel)
                                nc.vector.tensor_mul(
                                    atf[0][0:64, 0:qw], av[i][0:64, 0:qw], rec[:, 0:qw]
                                )
                            else:
                                nc.vector.reciprocal_approx_fast(
                                    rec[:, 0:qw], av[i][0:64, 0:qw]
                                )
                                # out at base partition 64 with inputs at 0 is
                                # fine for TensorTensor (not for stt below)
                                nc.vector.tensor_mul(
                                    atf[0][64:128, 0:qw], sel, rec[:, 0:qw]
                                )

                    def at_split():
                        # per half so every AP in the stt shares its start
                        # partition (verifier requirement)
                        for i in range(2):
                            r0 = i * 64
                            hi = ATh[r0 : r0 + 64, hp, q0 : q0 + qw]
                            src_h = atf[0][r0 : r0 + 64, 0:qw]
                            nc.vector.tensor_copy(hi, src_h)
                            nc.vector.scalar_tensor_tensor(
                                ATl[r0 : r0 + 64, hp, q0 : q0 + qw],
                                src_h,
                                0.0,
                                hi,
                                ALU.add,
                                ALU.subtract,
                            )

                    blk["scores"] = scores
                    blk["av"] = emit_av
                    blk["realign_copy"] = realign_copy
                    blk["realign"] = realign
                    blk["at_split"] = at_split
                    return blk

                def outproj_chain(tt, half, wo_p):
                    c0 = half * 512
                    ps = ppj.tile([128, 512], F32, tag="ps", bufs=2)
                    tsl = slice(tt * 128, (tt + 1) * 128)
                    aa = (ATh, ATl)
                    ww = wo_p
                    k = 0
                    for tl, tr in ((0, 0), (1, 0), (0, 1)):
                        for j in range(EP):
                            nc.tensor.matmul(
                                ps,
                                lhsT=aa[tl][:, 2 * j : 2 * j + 2, tsl],
                                rhs=ww[tr][:, 2 * j : 2 * j + 2, :],
                                start=(k == 0),
                                stop=(k == 3 * EP - 1),
                                perf_mode=DR,
                            )
                            k += 1
                    osb = p3.tile([128, 512], BF16, tag="osb", bufs=2)
                    nc.vector.tensor_add(osb, ps, boB[:, c0 : c0 + 512])
                    nc.sync.dma_start(
                        out=out[tt * 128 : (tt + 1) * 128, c0 : c0 + 512], in_=osb
                    )

                # --- prologue. DMA device is serial, so emission order is
                # arrival order; order strictly by first need, with chains
                # emitted as soon as their data is in flight. Block 0 starts
                # after just k0 tb0/tb1, six v0 chains, and q0 tb0; the
                # remaining block-0 prerequisites stream in as its inner work.
                wp_k0 = alloc_panel(E)
                wp_q0 = alloc_panel(0)
                dma_panel(wp_k0, E, 0, 0, 128)
                dma_panel(wp_k0, E, 1, 0, 128)
                dma_x(0)
                dma_panel(wp_q0, 0, 0, 0, 128)
                dma_panel(wp_q0, 0, 1, 0, 128)
                wp_v0 = load_panel(2 * E)
                dma_x(1)
                dma_panel(wp_k0, E, 0, 128, 384)
                dma_panel(wp_k0, E, 1, 128, 384)
                dma_panel(wp_q0, 0, 0, 128, 384)
                dma_panel(wp_q0, 0, 1, 128, 384)
                dma_x(2)
                dma_x(3)
                kq_chain(wp_k0, "k", 0, 0, 0)
                kq_chain(wp_q0, "q", 0, 0, 0)

                pss = _att_es.enter_context(
                    tc.tile_pool(name="ps_s", bufs=2, space="PSUM")
                )
                pav = _att_es.enter_context(
                    tc.tile_pool(name="ps_av", bufs=2, space="PSUM")
                )
                # --- chain schedule. V panel 0 feeds block 0 just-in-time
                # (v_chain(kt) emitted at iteration kt); V panel 1 likewise
                # splits across blocks 7/8. K/Q coltile ct of panel p feeds
                # head pair hp = 4p+ct, consumed from block 2hp on — each
                # chain group is emitted in an earlier block.
                wp_v1 = [None]
                wp_k1 = [None]
                wp_q1 = [None]
                wo_panels = [None, None]

                def kq_group(kind, panel, ct):
                    wp = {("k", 0): wp_k0, ("q", 0): wp_q0}.get((kind, panel))
                    n = 4 if kind == "k" else 2
                    if wp is not None:
                        return [
                            (lambda t=tb: kq_chain(wp, kind, panel, ct, t))
                            for tb in range(n)
                        ]
                    box = wp_k1 if kind == "k" else wp_q1
                    return [
                        (lambda t=tb: kq_chain(box[0], kind, panel, ct, t))
                        for tb in range(n)
                    ]

                def spread(chs):
                    # spread chain closures over kt slots 3..14, clear of the
                    # deferred realign (kt1) and at-split (kt3) DVE bursts and
                    # their ppj-ring selects
                    n = len(chs)
                    slots = [12 + (i * 4) // n for i in range(n)]
                    inner = {}
                    for s, ch in zip(slots, chs):
                        inner.setdefault(s, []).append(ch)
                    return inner

                def v1_group(tts):
                    return [(lambda t=tt: v_chain(wp_v1[0], 1, t)) for tt in tts]

                # per-block inner chain assignments (block -> kt -> chains),
                # balanced so no block carries more than ~4 chains beyond the
                # attention work, with every group emitted before its
                # consumer block (K/Q ct of panel p -> block 2*(4p+ct);
                # V panel 1 -> block 8).
                k0g = {c: kq_group("k", 0, c) for c in (1, 2, 3)}
                q0g = {c: kq_group("q", 0, c) for c in (1, 2, 3)}
                k1g = {c: kq_group("k", 1, c) for c in (0, 1, 2, 3)}
                q1g = {c: kq_group("q", 1, c) for c in (0, 1, 2, 3)}
                b0_inner = {
                    tt + 1: [lambda t=tt: v_chain(wp_v0, 0, t)] for tt in range(15)
                }
                b0_inner[15].append(lambda: v_chain(wp_v0, 0, 15))
                b0_inner[3] = b0_inner.get(3, []) + [
                    lambda: kq_chain(wp_k0, "k", 0, 0, 1)
                ]
                b0_inner[6].append(lambda: kq_chain(wp_k0, "k", 0, 0, 2))
                b0_inner[10].append(lambda: kq_chain(wp_k0, "k", 0, 0, 3))
                b0_inner[12].append(lambda: kq_chain(wp_q0, "q", 0, 0, 1))
                binner = {
                    0: b0_inner,
                    1: spread(k0g[1] + q0g[1]),
                    2: spread(k0g[2]),
                    3: spread(q0g[2] + v1_group(range(0, 2))),
                    4: spread(k0g[3]),
                    5: spread(q0g[3] + v1_group(range(2, 4))),
                    6: spread(k1g[0] + q1g[0]),
                    7: spread(v1_group(range(4, 10))),
                    8: {tt: [ch] for tt, ch in zip(range(10, 16), v1_group(range(10, 16)))},
                    9: spread(k1g[1] + q1g[1][:1]),
                    10: spread(q1g[1][1:] + k1g[2]),
                    11: spread(q1g[2] + k1g[3][:2]),
                    12: spread(k1g[3][2:] + q1g[3][:1]),
                    13: spread(q1g[3][1:]),
                    15: {
                        8 + i: [
                            (lambda t=tt, h=half: outproj_chain(t, h, wo_panels[h]))
                        ]
                        for i, (tt, half) in enumerate(
                            (t, h) for t in range(4) for h in range(2)
                        )
                    },
                }

                blocks = [(hp, qb * 512, 512) for hp in range(HP) for qb in range(QB)]
                pending = None
                prev = None  # (blk, kt) still owing its AV
                for b, (hp, q0, qw) in enumerate(blocks):
                    if b == 1:
                        # 4-slot wp ring: v1 lands in a fresh slot
                        wp_v1[0] = load_panel(2 * E + 512)
                    if b == 6:
                        # k1/q1 reuse k0/q0 slots; every k0/q0 reader has
                        # been emitted by now, so the anti-deps resolve.
                        wp_k1[0] = load_panel(E + 512)
                        wp_q1[0] = load_panel(512)
                    if b == 12:
                        wo_panels[0] = load_wo_panel(0)
                        wo_panels[1] = load_wo_panel(512)
                    blk = make_block(hp, q0, qw)
                    inner = binner.get(b, {})
                    for kt in range(KT):
                        blk["scores"](kt)
                        if kt == 1 and pending is not None:
                            pending[1]()
                        if kt == 7 and pending is not None:
                            pending[2]()
                        for ch in inner.get(kt, ()):
                            ch()
                        if prev is not None:
                            prev[0]["av"](prev[1])
                            if kt == 0 and pending is not None:
                                pending[0]()
                        prev = (blk, kt)
                    pending = (blk["realign_copy"], blk["realign"], blk["at_split"])
                prev[0]["av"](prev[1])
                pending[0]()
                pending[1]()
                pending[2]()
                for tt in range(4, 8):
                    for half in range(2):
                        outproj_chain(tt, half, wo_panels[half])
                _att_es.close()


_NC = None


def _get_program():
    global _NC
    if _NC is None:
        _NC = build_program()
    return _NC


def _split8(a):
    hi = a.astype(E4M3)
    lo = (a - hi.astype(np.float32)).astype(E4M3)
    return hi, lo


def make_in_maps(x, Wqkv, bqkv, Wo, bo):
    Wqkv = np.asarray(Wqkv, np.float32)
    bqkv = np.asarray(bqkv, np.float32)
    Wo = np.asarray(Wo, np.float32)
    bo = np.asarray(bo, np.float32)
    x = np.asarray(x, np.float32)

    wqh = Wqkv.astype(E4M3)
    wql = (Wqkv - wqh.astype(np.float32)).astype(E5M2)
    woh = Wo.astype(E4M3)
    wol = (Wo - woh.astype(np.float32)).astype(E5M2)
    bqt = np.ascontiguousarray(bqkv[:E].reshape(ET, 128).T)  # [128, 8]
    bob = bqkv[2 * E :] @ Wo + bo  # folded V-bias + out-bias
    w = {
        "wqh": np.ascontiguousarray(wqh),
        "wql": np.ascontiguousarray(wql),
        "woh": np.ascontiguousarray(woh),
        "wol": np.ascontiguousarray(wol),
        "bqt": bqt.astype(np.float32),
        "bob": bob.astype(np.float32),
    }
    in_maps = []
    for c in range(N_CORES):
        b, s = divmod(c, 2)
        xb = x[b]
        if s == 1:
            xb = np.roll(xb, -NQ, axis=0)
        xT = np.ascontiguousarray(xb.T).reshape(ET, 128, SEQ)
        xh, xl = _split8(xT)
        in_maps.append(
            {"xth": np.ascontiguousarray(xh), "xtl": np.ascontiguousarray(xl), **w}
        )
    return in_maps


def gather_out(results):
    out = np.empty((4, SEQ, E), np.float32)
    for c in range(N_CORES):
        b, s = divmod(c, 2)
        out[b, s * NQ : (s + 1) * NQ] = results[c]["out"].astype(np.float32)
    return out


def kernel(x, Wqkv, bqkv, Wo, bo):
    from concourse.bass_utils import run_bass_kernel_spmd

    nc = _get_program()
    in_maps = make_in_maps(x, Wqkv, bqkv, Wo, bo)
    res = run_bass_kernel_spmd(nc, in_maps, core_ids=list(range(N_CORES)))
    return gather_out(res.results)


# revision 6
# speedup vs baseline: 1.0615x; 1.0524x over previous
"""Trainium2 Bass kernel for a 16-head MHA layer (batch 4, seq 2048, embed 1024).

Sharding: 8 cores; core c handles batch c//2 and query-token half c%2.
Each core receives its batch's x rotated so its 1024 query tokens sit in rows
0:1024 (attention is permutation-invariant over key order). K/V are computed
over the full sequence on-core; weights are replicated; no collectives.

Numerics/compute strategy (fp8 DoubleRow on the PE at 0.5 cycles/row):
- Host passes x pre-transposed and split hi/lo: x = e4m3(x) + e4m3(residual),
  and Wqkv/Wo split as e4m3(W) + e5m2(residual) (the residual of the tiny
  uniform weights underflows e4m3's subnormals, hence e5m2).
- QKV projections: 3-term hi/lo DR chains (xh*Wh + xl*Wh + xh*Wl), e-chunk
  pairs in the two DR slots.
- Scores: DR with (K_hi, K_lo) stationary slots against a stride-0-duplicated
  e4m3 Q. Only the Q rounding survives to the output and the softmax damps it.
- The softmax denominator comes free from ones-columns in the AV stationary
  tile; exp runs on ACT straight out of PSUM; AV stays bf16.
- Out-projection: 3-term DR over an e4m3/e5m2 split of the attention output.
- The K bias is dropped (additive per-query constants are softmax-invariant);
  the V bias and output bias fold host-side into one bias row.

Schedule: a flat software pipeline over (block, kt) — AV lags scores/exp by
one step and crosses block boundaries, the realign/AT-split DVE work of each
block is deferred into the next block, and projection chains are packed into
the back half (kts 12..15) of earlier blocks so the exp stream stays hot.
"""

import sys

for _p in ("/opt/trn_rl_repo",):
    if _p not in sys.path:
        sys.path.insert(0, _p)

import numpy as np
import ml_dtypes

import concourse.bass as bass  # noqa: E402
import concourse.mybir as mybir  # noqa: E402
import concourse.tile as tile  # noqa: E402
from concourse import bacc  # noqa: E402
from concourse.masks import make_identity  # noqa: E402

SEQ = 2048
E = 1024
H = 16
D = 64
NQ = 1024  # query tokens per core
N_CORES = 8

F32 = mybir.dt.float32
BF16 = mybir.dt.bfloat16
FP8 = mybir.dt.float8e4
FP8E5 = mybir.dt.float8e5
AF = mybir.ActivationFunctionType
DR = mybir.MatmulPerfMode.DoubleRow
ALU = mybir.AluOpType

E4M3 = ml_dtypes.float8_e4m3
E5M2 = ml_dtypes.float8_e5m2

ET = E // 128  # 8 e-chunks
EP = ET // 2  # 4 e-chunk pairs (DoubleRow)
TT = SEQ // 128  # 16 token tiles
QB = NQ // 512  # 2 query blocks
KT = SEQ // 128  # 16 key tiles
HP = H // 2  # 8 head pairs


def build_program():
    nc = bacc.Bacc(trn_type="TRN2", target_bir_lowering=False, debug=False)

    # x transposed: [e-chunk, 128 e-rows, token] fp8 hi/lo
    xth = nc.dram_tensor("xth", [ET, 128, SEQ], FP8, kind="ExternalInput").ap()
    xtl = nc.dram_tensor("xtl", [ET, 128, SEQ], FP8, kind="ExternalInput").ap()
    wqh = nc.dram_tensor("wqh", [E, 3 * E], FP8, kind="ExternalInput").ap()
    wql = nc.dram_tensor("wql", [E, 3 * E], FP8E5, kind="ExternalInput").ap()
    woh = nc.dram_tensor("woh", [E, E], FP8, kind="ExternalInput").ap()
    wol = nc.dram_tensor("wol", [E, E], FP8E5, kind="ExternalInput").ap()
    # Q bias transposed: column g holds bqkv[g*128:(g+1)*128] for g in 0..7
    bqt = nc.dram_tensor("bqt", [128, ET], F32, kind="ExternalInput").ap()
    # folded output bias row: bqkv_v @ Wo + bo
    bob = nc.dram_tensor("bob", [E], F32, kind="ExternalInput").ap()
    out = nc.dram_tensor("out", [NQ, E], BF16, kind="ExternalOutput").ap()

    with tile.TileContext(nc) as tc:
        _body(nc, tc, xth, xtl, wqh, wql, woh, wol, bqt, bob, out)

    nc.compile()
    return nc


def _body(nc, tc, xth, xtl, wqh, wql, woh, wol, bqt, bob, out):
    from contextlib import ExitStack

    es = ExitStack()
    with es:
        pc = es.enter_context(tc.tile_pool(name="const", bufs=1))
        pat = es.enter_context(tc.tile_pool(name="at", bufs=1))

        # --- constants -------------------------------------------------
        ident = pc.tile([128, 128], BF16, tag="ident")
        make_identity(nc, ident)
        bqtS = pc.tile([128, ET], F32, tag="bqtS")
        nc.sync.dma_start(out=bqtS, in_=bqt)
        boB = pc.tile([128, E], F32, tag="boB")
        bob_bcast = bass.AP(
            tensor=bob.tensor, offset=bob.offset, ap=[[0, 128]] + bob.ap
        )
        nc.gpsimd.dma_start(out=boB, in_=bob_bcast)

        # attn output (transposed), fp8 hi/lo pair for the DR out-proj;
        # chunk p holds heads 2p (rows 0:64) and 2p+1 (rows 64:128)
        ATh = pat.tile([128, HP, NQ], FP8, tag="ath")
        ATl = pat.tile([128, HP, NQ], FP8E5, tag="atl")

        with (
            tc.tile_pool(name="kqv", bufs=1) as pkqv,
            tc.tile_pool(name="xT", bufs=1) as pxt,
            tc.tile_pool(name="wpan", bufs=4) as pw,
            tc.tile_pool(name="wo", bufs=1) as pwo,
            tc.tile_pool(name="ph3", bufs=3) as p3,
        ):
            # K^T in fp8 hi/lo slots: KT2[hp][:, 0, :] = hi, [:, 1, :] = lo
            KT2 = [
                pkqv.tile([128, 2, SEQ], FP8, tag=f"kt{i}", name=f"ktt{i}")
                for i in range(ET)
            ]
            QT8 = [
                pkqv.tile([128, NQ], FP8, tag=f"qt{i}", name=f"qtt{i}")
                for i in range(ET)
            ]
            # V in AV-stationary layout, e4m3 hi + e5m2 lo: per key-tile,
            # 8 head pairs of [V_h0 | ones | V_h1]; the ones block gives the
            # denominator for free (hi slot ones=1, lo slot ones=0).
            VOh = pkqv.tile([128, TT, HP, 192], FP8, tag="voh")
            VOl = pkqv.tile([128, TT, HP, 192], FP8E5, tag="vol")
            nc.vector.memset(VOl, 0.0)
            xh = pxt.tile([128, ET, SEQ], FP8, tag="xh")
            xl = pxt.tile([128, ET, SEQ], FP8, tag="xl")

            def dma_x_part(tb, which):
                # one hi/lo token-block slice (4KB/partition — big enough
                # that the 625ns HWDGE trigger hides under the transfer)
                t, src = (xh, xth) if which == 0 else (xl, xtl)
                nc.sync.dma_start(
                    out=t[:, :, tb * 512 : (tb + 1) * 512],
                    in_=bass.AP(
                        tensor=src.tensor,
                        offset=src.offset + tb * 512,
                        ap=[[SEQ, 128], [128 * SEQ, ET], [1, 512]],
                    ),
                )

            def dma_x(tb):
                dma_x_part(tb, 0)
                dma_x_part(tb, 1)

            def alloc_panel(pc0):
                return [
                    pw.tile(
                        [128, ET, 512],
                        FP8 if idx == 0 else FP8E5,
                        tag=f"wp{idx}",
                        name=f"wp{idx}_{pc0}",
                    )
                    for idx in range(2)
                ]

            def dma_panel(tiles, pc0, which, c0=0, cn=512):
                src = (wqh, wql)[which]
                nc.sync.dma_start(
                    out=tiles[which][:, :, c0 : c0 + cn],
                    in_=bass.AP(
                        tensor=src.tensor,
                        offset=src.offset + pc0 + c0,
                        ap=[[3 * E, 128], [3 * E * 128, ET], [1, cn]],
                    ),
                )

            def load_panel(pc0):
                tiles = alloc_panel(pc0)
                dma_panel(tiles, pc0, 0)
                dma_panel(tiles, pc0, 1)
                return tiles

            def load_wo_panel(pc0):
                tiles = []
                for idx, src_w in enumerate((woh, wol)):
                    wp = pwo.tile(
                        [128, ET, 512],
                        FP8 if idx == 0 else FP8E5,
                        tag=f"wo{idx}_{pc0}",
                        name=f"wo{idx}_{pc0}",
                    )
                    nc.sync.dma_start(
                        out=wp,
                        in_=bass.AP(
                            tensor=src_w.tensor,
                            offset=src_w.offset + pc0,
                            ap=[[E, 128], [E * 128, ET], [1, 512]],
                        ),
                    )
                    tiles.append(wp)
                return tiles

            def dup2(ap_base):
                # [P, N] -> [P, 2, N] with stride-0 middle dim (shared slot)
                return bass.AP(
                    tensor=ap_base.tensor,
                    offset=ap_base.offset,
                    ap=[ap_base.ap[0], [0, 2], ap_base.ap[1]],
                )

            with tc.tile_pool(name="ps_proj", bufs=1, space="PSUM") as ppj:
                from contextlib import ExitStack as _ES2

                _att_es = _ES2()

                def dr_chain(ps, lhs_of, rhs_of, nterm=3):
                    # 3-term hi/lo fp8 chain: hi*hi, lo*hi, hi*lo over 4
                    # e-chunk pairs each; all accumulate into one PSUM tile.
                    terms = [(0, 0), (1, 0), (0, 1)][:nterm]
                    n = len(terms) * EP
                    k = 0
                    for tl, tr in terms:
                        for j in range(EP):
                            nc.tensor.matmul(
                                ps,
                                lhsT=lhs_of(tl, j),
                                rhs=rhs_of(tr, j),
                                start=(k == 0),
                                stop=(k == n - 1),
                                perf_mode=DR,
                            )
                            k += 1

                def v_chain(wp, panel, tt):
                    # one V-proj output tile -> VO pair layout (+ ones memset)
                    wph, wpl = wp
                    p0 = panel * 4
                    ps = ppj.tile([128, 512], F32, tag="ps", bufs=2)
                    xx = (xh, xl)
                    ww = (wph, wpl)
                    dr_chain(
                        ps,
                        lambda tl, j: xx[tl][:, 2 * j : 2 * j + 2, tt * 128 : (tt + 1) * 128],
                        lambda tr, j: ww[tr][:, 2 * j : 2 * j + 2, :],
                    )
                    ps3 = ps.rearrange("p (pr d) -> p pr d", d=128)
                    for c0, c1, s0, s1 in ((0, 64, 0, 64), (128, 192, 64, 128)):
                        hi = VOh[:, tt, p0 : p0 + 4, c0:c1]
                        nc.vector.tensor_copy(hi, ps3[:, :, s0:s1])
                        nc.vector.scalar_tensor_tensor(
                            VOl[:, tt, p0 : p0 + 4, c0:c1],
                            ps3[:, :, s0:s1],
                            0.0,
                            hi,
                            ALU.add,
                            ALU.subtract,
                        )
                    if panel == 0:
                        nc.vector.memset(VOh[:, tt, :, 64:128], 1.0)

                def kq_chain(wp, kind, panel, ct, tb):
                    # one K^T/Q^T-proj output tile (Q gets bias; K needs none —
                    # a per-query additive constant is softmax-invariant)
                    wph, wpl = wp
                    gct = panel * 4 + ct
                    ps = ppj.tile([128, 512], F32, tag="ps", bufs=2)
                    xx = (xh, xl)
                    ww = (wph, wpl)
                    dr_chain(
                        ps,
                        lambda tl, j: ww[tl][:, 2 * j : 2 * j + 2, ct * 128 : (ct + 1) * 128],
                        lambda tr, j: xx[tr][:, 2 * j : 2 * j + 2, tb * 512 : (tb + 1) * 512],
                        # lhs/rhs term roles are swapped relative to v_chain:
                        # terms needed are (Wh,xh), (Wh,xl), (Wl,xh)
                    )
                    tsl = slice(tb * 512, (tb + 1) * 512)
                    if kind == "q":
                        nc.vector.tensor_scalar_add(
                            QT8[gct][:, tsl], ps, bqtS[:, gct : gct + 1]
                        )
                    else:
                        nc.vector.tensor_copy(KT2[gct][:, 0, tsl], ps)
                        nc.vector.scalar_tensor_tensor(
                            KT2[gct][:, 1, tsl],
                            ps,
                            0.0,
                            KT2[gct][:, 0, tsl],
                            ALU.add,
                            ALU.subtract,
                        )

                def make_block(hp, q0, qw):
                    # Flat cross-block software pipeline: the caller emits
                    # scores/exp for (b, kt) and the AV of the PREVIOUS
                    # (b, kt) pair, so the exp stream never queues behind a
                    # block boundary. qw is the query width (256 for the
                    # split final block).
                    av = [
                        pav.tile([128, 512], F32, tag="av", name=f"av{hp}_{q0}_{i}")
                        for i in range(2)
                    ]
                    pts = [None] * (KT // 2)
                    cur = [None]
                    blk = {}

                    def scores(kt):
                        k0 = kt * 128
                        ps_s = pss.tile([128, 1024], F32, tag="ps_s")
                        for i in range(2):
                            r0 = i * 64
                            nc.tensor.matmul(
                                ps_s[:, i * qw : (i + 1) * qw],
                                lhsT=KT2[hp][r0 : r0 + 64, :, k0 : k0 + 128],
                                rhs=dup2(QT8[hp][r0 : r0 + 64, q0 : q0 + qw]),
                                start=True,
                                stop=True,
                                perf_mode=DR,
                            )
                        if kt % 2 == 0:
                            cur[0] = p3.tile(
                                [128, 2, 1024],
                                FP8,
                                tag="pt",
                                bufs=4,
                                name=f"pt{hp}_{q0}_{kt}",
                            )
                            pts[kt // 2] = cur[0]
                        nc.scalar.activation(
                            cur[0][:, kt % 2, 0 : 2 * qw],
                            ps_s[:, 0 : 2 * qw],
                            AF.Exp,
                            scale=0.125,
                        )

                    def emit_av(p):
                        # one kt-pair per DR instruction; the e5m2 lo slot's
                        # ones region is zero so the denominator stays exact
                        pt2 = pts[p]
                        for i in range(2):
                            # i=0: rows 0:64 = V.T @ P, rows 64:128 = denom
                            # i=1: rows 0:64 = denom, rows 64:128 = V.T @ P
                            for w, VOx in ((0, VOh), (1, VOl)):
                                nc.tensor.matmul(
                                    av[i][:, 0:qw],
                                    lhsT=VOx[
                                        :, 2 * p : 2 * p + 2, hp, 64 * i : 64 * i + 128
                                    ],
                                    rhs=pt2[:, :, i * qw : (i + 1) * qw],
                                    start=(p == 0 and w == 0),
                                    stop=(p == KT // 2 - 1 and w == 1),
                                    perf_mode=DR,
                                )
                        pts[p] = None

                    avsbs = []
                    atf = [None]

                    def realign_copy():
                        # bf16 copies of both AV psums, emitted right after the
                        # final AV so the DVE is done before the PE selects
                        # reach the queue head (a PE-queue stall otherwise).
                        for i in range(2):
                            av_sb = p3.tile([128, 512], BF16, tag="avsb", bufs=2)
                            nc.vector.tensor_copy(av_sb[:, 0:qw], av[i][:, 0:qw])
                            avsbs.append(av_sb)

                    def realign():
                        # DVE ops need all inputs at base partition 0; realign
                        # the half at rows 64:128 via a PE select-matmul.
                        for i in range(2):
                            rec = p3.tile([64, 512], F32, tag="rec", bufs=2)
                            av_sb = avsbs[i]
                            # selects share the double-buffered chain-psum ring
                            sel_t = ppj.tile([128, 512], F32, tag="ps", bufs=2)
                            sel = sel_t[0:64, 0:qw]
                            nc.tensor.matmul(
                                sel, lhsT=ident[:, 64:128], rhs=av_sb[:, 0:qw],
                                start=True, stop=True,
                            )
                            if i == 0:
                                atf[0] = p3.tile([128, 512], BF16, tag="atf", bufs=2, name=f"atf{hp}_{q0}")
                                nc.vector.reciprocal_approx_fast(rec[:, 0:qw], sel)
                                nc.vector.tensor_mul(
                                    atf[0][0:64, 0:qw], av[i][0:64, 0:qw], rec[:, 0:qw]
                                )
                            else:
                                nc.vector.reciprocal_approx_fast(
                                    rec[:, 0:qw], av[i][0:64, 0:qw]
                                )
                                # out at base partition 64 with inputs at 0 is
                                # fine for TensorTensor (not for stt below)
                                nc.vector.tensor_mul(
                                    atf[0][64:128, 0:qw], sel, rec[:, 0:qw]
                                )

                    def at_split():
                        # per half so every AP in the stt shares its start
                        # partition (verifier requirement)
                        for i in range(2):
                            r0 = i * 64
                            hi = ATh[r0 : r0 + 64, hp, q0 : q0 + qw]
                            src_h = atf[0][r0 : r0 + 64, 0:qw]
                            nc.vector.tensor_copy(hi, src_h)
                            nc.vector.scalar_tensor_tensor(
                                ATl[r0 : r0 + 64, hp, q0 : q0 + qw],
                                src_h,
                                0.0,
                                hi,
                                ALU.add,
                                ALU.subtract,
                            )

                    blk["scores"] = scores
                    blk["av"] = emit_av
                    blk["realign_copy"] = realign_copy
                    blk["realign"] = realign
                    blk["at_split"] = at_split
                    return blk

                def outproj_chain(tt, half, wo_p):
                    c0 = half * 512
                    ps = ppj.tile([128, 512], F32, tag="ps", bufs=2)
                    tsl = slice(tt * 128, (tt + 1) * 128)
                    aa = (ATh, ATl)
                    ww = wo_p
                    k = 0
                    for tl, tr in ((0, 0), (1, 0), (0, 1)):
                        for j in range(EP):
                            nc.tensor.matmul(
                                ps,
                                lhsT=aa[tl][:, 2 * j : 2 * j + 2, tsl],
                                rhs=ww[tr][:, 2 * j : 2 * j + 2, :],
                                start=(k == 0),
                                stop=(k == 3 * EP - 1),
                                perf_mode=DR,
                            )
                            k += 1
                    osb = p3.tile([128, 512], BF16, tag="osb", bufs=2)
                    nc.vector.tensor_add(osb, ps, boB[:, c0 : c0 + 512])
                    nc.sync.dma_start(
                        out=out[tt * 128 : (tt + 1) * 128, c0 : c0 + 512], in_=osb
                    )

                # --- prologue. DMA device is serial, so emission order is
                # arrival order; order strictly by first need, with chains
                # emitted as soon as their data is in flight. Block 0 starts
                # after just k0 tb0/tb1, six v0 chains, and q0 tb0; the
                # remaining block-0 prerequisites stream in as its inner work.
                wp_k0 = alloc_panel(E)
                wp_q0 = alloc_panel(0)
                dma_panel(wp_k0, E, 0, 0, 128)
                dma_panel(wp_k0, E, 1, 0, 128)
                dma_x(0)
                dma_panel(wp_q0, 0, 0, 0, 128)
                dma_panel(wp_q0, 0, 1, 0, 128)
                wp_v0 = load_panel(2 * E)
                dma_x(1)
                dma_panel(wp_k0, E, 0, 128, 384)
                dma_panel(wp_k0, E, 1, 128, 384)
                dma_panel(wp_q0, 0, 0, 128, 384)
                dma_panel(wp_q0, 0, 1, 128, 384)
                dma_x(2)
                dma_x(3)
                kq_chain(wp_k0, "k", 0, 0, 0)
                kq_chain(wp_q0, "q", 0, 0, 0)

                pss = _att_es.enter_context(
                    tc.tile_pool(name="ps_s", bufs=2, space="PSUM")
                )
                pav = _att_es.enter_context(
                    tc.tile_pool(name="ps_av", bufs=2, space="PSUM")
                )
                # --- chain schedule. V panel 0 feeds block 0 just-in-time
                # (v_chain(kt) emitted at iteration kt); V panel 1 likewise
                # splits across blocks 7/8. K/Q coltile ct of panel p feeds
                # head pair hp = 4p+ct, consumed from block 2hp on — each
                # chain group is emitted in an earlier block.
                wp_v1 = [None]
                wp_k1 = [None]
                wp_q1 = [None]
                wo_panels = [None, None]

                def kq_group(kind, panel, ct):
                    wp = {("k", 0): wp_k0, ("q", 0): wp_q0}.get((kind, panel))
                    n = 4 if kind == "k" else 2
                    if wp is not None:
                        return [
                            (lambda t=tb: kq_chain(wp, kind, panel, ct, t))
                            for tb in range(n)
                        ]
                    box = wp_k1 if kind == "k" else wp_q1
                    return [
                        (lambda t=tb: kq_chain(box[0], kind, panel, ct, t))
                        for tb in range(n)
                    ]

                def spread(chs):
                    # spread chain closures over kt slots 3..14, clear of the
                    # deferred realign (kt1) and at-split (kt3) DVE bursts and
                    # their ppj-ring selects
                    n = len(chs)
                    slots = [12 + (i * 4) // n for i in range(n)]
                    inner = {}
                    for s, ch in zip(slots, chs):
                        inner.setdefault(s, []).append(ch)
                    return inner

                def v1_group(tts):
                    return [(lambda t=tt: v_chain(wp_v1[0], 1, t)) for tt in tts]

                # per-block inner chain assignments (block -> kt -> chains),
                # balanced so no block carries more than ~4 chains beyond the
                # attention work, with every group emitted before its
                # consumer block (K/Q ct of panel p -> block 2*(4p+ct);
                # V panel 1 -> block 8).
                k0g = {c: kq_group("k", 0, c) for c in (1, 2, 3)}
                q0g = {c: kq_group("q", 0, c) for c in (1, 2, 3)}
                k1g = {c: kq_group("k", 1, c) for c in (0, 1, 2, 3)}
                q1g = {c: kq_group("q", 1, c) for c in (0, 1, 2, 3)}
                b0_inner = {
                    tt + 1: [lambda t=tt: v_chain(wp_v0, 0, t)] for tt in range(15)
                }
                b0_inner[15].append(lambda: v_chain(wp_v0, 0, 15))
                b0_inner[3] = b0_inner.get(3, []) + [
                    lambda: kq_chain(wp_k0, "k", 0, 0, 1)
                ]
                b0_inner[6].append(lambda: kq_chain(wp_k0, "k", 0, 0, 2))
                b0_inner[10].append(lambda: kq_chain(wp_k0, "k", 0, 0, 3))
                b0_inner[12].append(lambda: kq_chain(wp_q0, "q", 0, 0, 1))
                binner = {
                    0: b0_inner,
                    1: spread(k0g[1] + q0g[1]),
                    2: spread(k0g[2]),
                    3: spread(q0g[2] + v1_group(range(0, 2))),
                    4: spread(k0g[3]),
                    5: spread(q0g[3] + v1_group(range(2, 4))),
                    6: spread(k1g[0] + q1g[0]),
                    7: spread(v1_group(range(4, 10))),
                    8: {tt: [ch] for tt, ch in zip(range(10, 16), v1_group(range(10, 16)))},
                    9: spread(k1g[1] + q1g[1][:1]),
                    10: spread(q1g[1][1:] + k1g[2]),
                    11: spread(q1g[2] + k1g[3][:2]),
                    12: spread(k1g[3][2:] + q1g[3][:1]),
                    13: spread(q1g[3][1:]),
                    15: {
                        8 + i: [
                            (lambda t=tt, h=half: outproj_chain(t, h, wo_panels[h]))
                        ]
                        for i, (tt, half) in enumerate(
                            (t, h) for t in range(4) for h in range(2)
                        )
                    },
                }

                blocks = [(hp, qb * 512, 512) for hp in range(HP) for qb in range(QB)]
                pending = None
                prev = None  # (blk, kt) still owing its AV
                for b, (hp, q0, qw) in enumerate(blocks):
                    if b == 1:
                        # 4-slot wp ring: v1 lands in a fresh slot
                        wp_v1[0] = load_panel(2 * E + 512)
                    if b == 6:
                        # k1/q1 reuse k0/q0 slots; every k0/q0 reader has
                        # been emitted by now, so the anti-deps resolve.
                        wp_k1[0] = load_panel(E + 512)
                        wp_q1[0] = load_panel(512)
                    if b == 12:
                        wo_panels[0] = load_wo_panel(0)
                        wo_panels[1] = load_wo_panel(512)
                    blk = make_block(hp, q0, qw)
                    inner = binner.get(b, {})
                    for kt in range(KT):
                        blk["scores"](kt)
                        if kt == 3 and pending is not None:
                            pending[1]()
                        if kt == 7 and pending is not None:
                            pending[2]()
                        for ch in inner.get(kt, ()):
                            ch()
                        if kt % 2 == 1:
                            if prev is not None:
                                prev[0]["av"](prev[1])
                                if prev[1] == KT // 2 - 1 and pending is not None:
                                    pending[0]()
                            prev = (blk, kt // 2)
                    pending = (blk["realign_copy"], blk["realign"], blk["at_split"])
                prev[0]["av"](prev[1])
                pending[0]()
                pending[1]()
                pending[2]()
                for tt in range(4, 8):
                    for half in range(2):
                        outproj_chain(tt, half, wo_panels[half])
                _att_es.close()


_NC = None


def _get_program():
    global _NC
    if _NC is None:
        _NC = build_program()
    return _NC


def _split8(a):
    hi = a.astype(E4M3)
    lo = (a - hi.astype(np.float32)).astype(E4M3)
    return hi, lo


def make_in_maps(x, Wqkv, bqkv, Wo, bo):
    Wqkv = np.asarray(Wqkv, np.float32)
    bqkv = np.asarray(bqkv, np.float32)
    Wo = np.asarray(Wo, np.float32)
    bo = np.asarray(bo, np.float32)
    x = np.asarray(x, np.float32)

    wqh = Wqkv.astype(E4M3)
    wql = (Wqkv - wqh.astype(np.float32)).astype(E5M2)
    woh = Wo.astype(E4M3)
    wol = (Wo - woh.astype(np.float32)).astype(E5M2)
    bqt = np.ascontiguousarray(bqkv[:E].reshape(ET, 128).T)  # [128, 8]
    bob = bqkv[2 * E :] @ Wo + bo  # folded V-bias + out-bias
    w = {
        "wqh": np.ascontiguousarray(wqh),
        "wql": np.ascontiguousarray(wql),
        "woh": np.ascontiguousarray(woh),
        "wol": np.ascontiguousarray(wol),
        "bqt": bqt.astype(np.float32),
        "bob": bob.astype(np.float32),
    }
    in_maps = []
    for c in range(N_CORES):
        b, s = divmod(c, 2)
        xb = x[b]
        if s == 1:
            xb = np.roll(xb, -NQ, axis=0)
        xT = np.ascontiguousarray(xb.T).reshape(ET, 128, SEQ)
        xh, xl = _split8(xT)
        in_maps.append(
            {"xth": np.ascontiguousarray(xh), "xtl": np.ascontiguousarray(xl), **w}
        )
    return in_maps


def gather_out(results):
    out = np.empty((4, SEQ, E), np.float32)
    for c in range(N_CORES):
        b, s = divmod(c, 2)
        out[b, s * NQ : (s + 1) * NQ] = results[c]["out"].astype(np.float32)
    return out


def kernel(x, Wqkv, bqkv, Wo, bo):
    from concourse.bass_utils import run_bass_kernel_spmd

    nc = _get_program()
    in_maps = make_in_maps(x, Wqkv, bqkv, Wo, bo)
    res = run_bass_kernel_spmd(nc, in_maps, core_ids=list(range(N_CORES)))
    return gather_out(res.results)


# revision 7
# speedup vs baseline: 1.0631x; 1.0015x over previous
"""Trainium2 Bass kernel for a 16-head MHA layer (batch 4, seq 2048, embed 1024).

Sharding: 8 cores; core c handles batch c//2 and query-token half c%2.
Each core receives its batch's x rotated so its 1024 query tokens sit in rows
0:1024 (attention is permutation-invariant over key order). K/V are computed
over the full sequence on-core; weights are replicated; no collectives.

Numerics/compute strategy (fp8 DoubleRow on the PE at 0.5 cycles/row):
- Host passes x pre-transposed and split hi/lo: x = e4m3(x) + e4m3(residual),
  and Wqkv/Wo split as e4m3(W) + e5m2(residual) (the residual of the tiny
  uniform weights underflows e4m3's subnormals, hence e5m2).
- QKV projections: 3-term hi/lo DR chains (xh*Wh + xl*Wh + xh*Wl), e-chunk
  pairs in the two DR slots.
- Scores: DR with (K_hi, K_lo) stationary slots against a stride-0-duplicated
  e4m3 Q. Only the Q rounding survives to the output and the softmax damps it.
- The softmax denominator comes free from ones-columns in the AV stationary
  tile; exp runs on ACT straight out of PSUM; AV stays bf16.
- Out-projection: 3-term DR over an e4m3/e5m2 split of the attention output.
- The K bias is dropped (additive per-query constants are softmax-invariant);
  the V bias and output bias fold host-side into one bias row.

Schedule: a flat software pipeline over (block, kt) — AV lags scores/exp by
one step and crosses block boundaries, the realign/AT-split DVE work of each
block is deferred into the next block, and projection chains are packed into
the back half (kts 12..15) of earlier blocks so the exp stream stays hot.
"""

import sys

for _p in ("/opt/trn_rl_repo",):
    if _p not in sys.path:
        sys.path.insert(0, _p)

import numpy as np
import ml_dtypes

import concourse.bass as bass  # noqa: E402
import concourse.mybir as mybir  # noqa: E402
import concourse.tile as tile  # noqa: E402
from concourse import bacc  # noqa: E402
from concourse.masks import make_identity  # noqa: E402

SEQ = 2048
E = 1024
H = 16
D = 64
NQ = 1024  # query tokens per core
N_CORES = 8

F32 = mybir.dt.float32
BF16 = mybir.dt.bfloat16
FP8 = mybir.dt.float8e4
FP8E5 = mybir.dt.float8e5
AF = mybir.ActivationFunctionType
DR = mybir.MatmulPerfMode.DoubleRow
ALU = mybir.AluOpType

E4M3 = ml_dtypes.float8_e4m3
E5M2 = ml_dtypes.float8_e5m2

ET = E // 128  # 8 e-chunks
EP = ET // 2  # 4 e-chunk pairs (DoubleRow)
TT = SEQ // 128  # 16 token tiles
QB = NQ // 512  # 2 query blocks
KT = SEQ // 128  # 16 key tiles
HP = H // 2  # 8 head pairs


def build_program():
    nc = bacc.Bacc(trn_type="TRN2", target_bir_lowering=False, debug=False)

    # x transposed: [e-chunk, 128 e-rows, token] fp8 hi/lo
    xth = nc.dram_tensor("xth", [ET, 128, SEQ], FP8, kind="ExternalInput").ap()
    xtl = nc.dram_tensor("xtl", [ET, 128, SEQ], FP8, kind="ExternalInput").ap()
    wqh = nc.dram_tensor("wqh", [E, 3 * E], FP8, kind="ExternalInput").ap()
    wql = nc.dram_tensor("wql", [E, 3 * E], FP8E5, kind="ExternalInput").ap()
    woh = nc.dram_tensor("woh", [E, E], FP8, kind="ExternalInput").ap()
    wol = nc.dram_tensor("wol", [E, E], FP8E5, kind="ExternalInput").ap()
    # Q bias transposed: column g holds bqkv[g*128:(g+1)*128] for g in 0..7
    bqt = nc.dram_tensor("bqt", [128, ET], F32, kind="ExternalInput").ap()
    # folded output bias row: bqkv_v @ Wo + bo
    bob = nc.dram_tensor("bob", [E], F32, kind="ExternalInput").ap()
    out = nc.dram_tensor("out", [NQ, E], BF16, kind="ExternalOutput").ap()

    with tile.TileContext(nc) as tc:
        _body(nc, tc, xth, xtl, wqh, wql, woh, wol, bqt, bob, out)

    nc.compile()
    return nc


def _body(nc, tc, xth, xtl, wqh, wql, woh, wol, bqt, bob, out):
    from contextlib import ExitStack

    es = ExitStack()
    with es:
        pc = es.enter_context(tc.tile_pool(name="const", bufs=1))
        pat = es.enter_context(tc.tile_pool(name="at", bufs=1))

        # --- constants -------------------------------------------------
        ident = pc.tile([128, 128], BF16, tag="ident")
        make_identity(nc, ident)
        bqtS = pc.tile([128, ET], F32, tag="bqtS")
        nc.sync.dma_start(out=bqtS, in_=bqt)
        boB = pc.tile([128, E], F32, tag="boB")
        bob_bcast = bass.AP(
            tensor=bob.tensor, offset=bob.offset, ap=[[0, 128]] + bob.ap
        )
        nc.gpsimd.dma_start(out=boB, in_=bob_bcast)

        # attn output (transposed), fp8 hi/lo pair for the DR out-proj;
        # chunk p holds heads 2p (rows 0:64) and 2p+1 (rows 64:128)
        ATh = pat.tile([128, HP, NQ], FP8, tag="ath")
        ATl = pat.tile([128, HP, NQ], FP8E5, tag="atl")

        with (
            tc.tile_pool(name="kqv", bufs=1) as pkqv,
            tc.tile_pool(name="xT", bufs=1) as pxt,
            tc.tile_pool(name="wpan", bufs=4) as pw,
            tc.tile_pool(name="wo", bufs=1) as pwo,
            tc.tile_pool(name="ph3", bufs=3) as p3,
        ):
            # K^T in fp8 hi/lo slots: KT2[hp][:, 0, :] = hi, [:, 1, :] = lo
            KT2 = [
                pkqv.tile([128, 2, SEQ], FP8, tag=f"kt{i}", name=f"ktt{i}")
                for i in range(ET)
            ]
            QT8 = [
                pkqv.tile([128, NQ], FP8, tag=f"qt{i}", name=f"qtt{i}")
                for i in range(ET)
            ]
            # V in AV-stationary layout, e4m3 hi + e5m2 lo: per key-tile,
            # 8 head pairs of [V_h0 | ones | V_h1]; the ones block gives the
            # denominator for free (hi slot ones=1, lo slot ones=0).
            VOh = pkqv.tile([128, TT, HP, 192], FP8, tag="voh")
            VOl = pkqv.tile([128, TT, HP, 192], FP8E5, tag="vol")
            xh = pxt.tile([128, ET, SEQ], FP8, tag="xh")
            xl = pxt.tile([128, ET, SEQ], FP8, tag="xl")

            def dma_x_part(tb, which):
                # one hi/lo token-block slice (4KB/partition — big enough
                # that the 625ns HWDGE trigger hides under the transfer)
                t, src = (xh, xth) if which == 0 else (xl, xtl)
                nc.sync.dma_start(
                    out=t[:, :, tb * 512 : (tb + 1) * 512],
                    in_=bass.AP(
                        tensor=src.tensor,
                        offset=src.offset + tb * 512,
                        ap=[[SEQ, 128], [128 * SEQ, ET], [1, 512]],
                    ),
                )

            def dma_x(tb):
                dma_x_part(tb, 0)
                dma_x_part(tb, 1)

            def alloc_panel(pc0):
                return [
                    pw.tile(
                        [128, ET, 512],
                        FP8 if idx == 0 else FP8E5,
                        tag=f"wp{idx}",
                        name=f"wp{idx}_{pc0}",
                    )
                    for idx in range(2)
                ]

            def dma_panel(tiles, pc0, which, c0=0, cn=512):
                src = (wqh, wql)[which]
                nc.sync.dma_start(
                    out=tiles[which][:, :, c0 : c0 + cn],
                    in_=bass.AP(
                        tensor=src.tensor,
                        offset=src.offset + pc0 + c0,
                        ap=[[3 * E, 128], [3 * E * 128, ET], [1, cn]],
                    ),
                )

            def load_panel(pc0):
                tiles = alloc_panel(pc0)
                dma_panel(tiles, pc0, 0)
                dma_panel(tiles, pc0, 1)
                return tiles

            def load_wo_panel(pc0):
                tiles = []
                for idx, src_w in enumerate((woh, wol)):
                    wp = pwo.tile(
                        [128, ET, 512],
                        FP8 if idx == 0 else FP8E5,
                        tag=f"wo{idx}_{pc0}",
                        name=f"wo{idx}_{pc0}",
                    )
                    nc.sync.dma_start(
                        out=wp,
                        in_=bass.AP(
                            tensor=src_w.tensor,
                            offset=src_w.offset + pc0,
                            ap=[[E, 128], [E * 128, ET], [1, 512]],
                        ),
                    )
                    tiles.append(wp)
                return tiles

            def dup2(ap_base):
                # [P, N] -> [P, 2, N] with stride-0 middle dim (shared slot)
                return bass.AP(
                    tensor=ap_base.tensor,
                    offset=ap_base.offset,
                    ap=[ap_base.ap[0], [0, 2], ap_base.ap[1]],
                )

            with tc.tile_pool(name="ps_proj", bufs=1, space="PSUM") as ppj:
                from contextlib import ExitStack as _ES2

                _att_es = _ES2()

                def dr_chain(ps, lhs_of, rhs_of, nterm=3):
                    # 3-term hi/lo fp8 chain: hi*hi, lo*hi, hi*lo over 4
                    # e-chunk pairs each; all accumulate into one PSUM tile.
                    terms = [(0, 0), (1, 0), (0, 1)][:nterm]
                    n = len(terms) * EP
                    k = 0
                    for tl, tr in terms:
                        for j in range(EP):
                            nc.tensor.matmul(
                                ps,
                                lhsT=lhs_of(tl, j),
                                rhs=rhs_of(tr, j),
                                start=(k == 0),
                                stop=(k == n - 1),
                                perf_mode=DR,
                            )
                            k += 1

                def v_chain(wp, panel, tt):
                    # one V-proj output tile -> VO pair layout (+ ones memset)
                    wph, wpl = wp
                    p0 = panel * 4
                    ps = ppj.tile([128, 512], F32, tag="ps", bufs=2)
                    xx = (xh, xl)
                    ww = (wph, wpl)
                    dr_chain(
                        ps,
                        lambda tl, j: xx[tl][:, 2 * j : 2 * j + 2, tt * 128 : (tt + 1) * 128],
                        lambda tr, j: ww[tr][:, 2 * j : 2 * j + 2, :],
                    )
                    ps3 = ps.rearrange("p (pr d) -> p pr d", d=128)
                    for c0, c1, s0, s1 in ((0, 64, 0, 64), (128, 192, 64, 128)):
                        hi = VOh[:, tt, p0 : p0 + 4, c0:c1]
                        nc.vector.tensor_copy(hi, ps3[:, :, s0:s1])
                        nc.vector.scalar_tensor_tensor(
                            VOl[:, tt, p0 : p0 + 4, c0:c1],
                            ps3[:, :, s0:s1],
                            0.0,
                            hi,
                            ALU.add,
                            ALU.subtract,
                        )
                    if panel == 0:
                        nc.vector.memset(VOh[:, tt, :, 64:128], 1.0)
                        nc.vector.memset(VOl[:, tt, :, 64:128], 0.0)

                def kq_chain(wp, kind, panel, ct, tb):
                    # one K^T/Q^T-proj output tile (Q gets bias; K needs none —
                    # a per-query additive constant is softmax-invariant)
                    wph, wpl = wp
                    gct = panel * 4 + ct
                    ps = ppj.tile([128, 512], F32, tag="ps", bufs=2)
                    xx = (xh, xl)
                    ww = (wph, wpl)
                    dr_chain(
                        ps,
                        lambda tl, j: ww[tl][:, 2 * j : 2 * j + 2, ct * 128 : (ct + 1) * 128],
                        lambda tr, j: xx[tr][:, 2 * j : 2 * j + 2, tb * 512 : (tb + 1) * 512],
                        # lhs/rhs term roles are swapped relative to v_chain:
                        # terms needed are (Wh,xh), (Wh,xl), (Wl,xh)
                    )
                    tsl = slice(tb * 512, (tb + 1) * 512)
                    if kind == "q":
                        nc.vector.tensor_scalar_add(
                            QT8[gct][:, tsl], ps, bqtS[:, gct : gct + 1]
                        )
                    else:
                        nc.vector.tensor_copy(KT2[gct][:, 0, tsl], ps)
                        nc.vector.scalar_tensor_tensor(
                            KT2[gct][:, 1, tsl],
                            ps,
                            0.0,
                            KT2[gct][:, 0, tsl],
                            ALU.add,
                            ALU.subtract,
                        )

                def make_block(hp, q0, qw):
                    # Flat cross-block software pipeline: the caller emits
                    # scores/exp for (b, kt) and the AV of the PREVIOUS
                    # (b, kt) pair, so the exp stream never queues behind a
                    # block boundary. qw is the query width (256 for the
                    # split final block).
                    av = [
                        pav.tile([128, 512], F32, tag="av", name=f"av{hp}_{q0}_{i}")
                        for i in range(2)
                    ]
                    pts = [None] * (KT // 2)
                    cur = [None]
                    blk = {}

                    def scores(kt):
                        k0 = kt * 128
                        ps_s = pss.tile([128, 1024], F32, tag="ps_s")
                        for i in range(2):
                            r0 = i * 64
                            nc.tensor.matmul(
                                ps_s[:, i * qw : (i + 1) * qw],
                                lhsT=KT2[hp][r0 : r0 + 64, :, k0 : k0 + 128],
                                rhs=dup2(QT8[hp][r0 : r0 + 64, q0 : q0 + qw]),
                                start=True,
                                stop=True,
                                perf_mode=DR,
                            )
                        if kt % 2 == 0:
                            cur[0] = p3.tile(
                                [128, 2, 1024],
                                FP8,
                                tag="pt",
                                bufs=4,
                                name=f"pt{hp}_{q0}_{kt}",
                            )
                            pts[kt // 2] = cur[0]
                        nc.scalar.activation(
                            cur[0][:, kt % 2, 0 : 2 * qw],
                            ps_s[:, 0 : 2 * qw],
                            AF.Exp,
                            scale=0.125,
                        )

                    def emit_av(p):
                        # one kt-pair per DR instruction; the e5m2 lo slot's
                        # ones region is zero so the denominator stays exact
                        pt2 = pts[p]
                        for i in range(2):
                            # i=0: rows 0:64 = V.T @ P, rows 64:128 = denom
                            # i=1: rows 0:64 = denom, rows 64:128 = V.T @ P
                            for w, VOx in ((0, VOh), (1, VOl)):
                                nc.tensor.matmul(
                                    av[i][:, 0:qw],
                                    lhsT=VOx[
                                        :, 2 * p : 2 * p + 2, hp, 64 * i : 64 * i + 128
                                    ],
                                    rhs=pt2[:, :, i * qw : (i + 1) * qw],
                                    start=(p == 0 and w == 0),
                                    stop=(p == KT // 2 - 1 and w == 1),
                                    perf_mode=DR,
                                )
                        pts[p] = None

                    avsbs = []
                    atf = [None]

                    def realign_copy():
                        # bf16 copies of both AV psums, emitted right after the
                        # final AV so the DVE is done before the PE selects
                        # reach the queue head (a PE-queue stall otherwise).
                        for i in range(2):
                            av_sb = p3.tile([128, 512], BF16, tag="avsb", bufs=2)
                            nc.vector.tensor_copy(av_sb[:, 0:qw], av[i][:, 0:qw])
                            avsbs.append(av_sb)

                    def realign():
                        # DVE ops need all inputs at base partition 0; realign
                        # the half at rows 64:128 via a PE select-matmul.
                        for i in range(2):
                            rec = p3.tile([64, 512], F32, tag="rec", bufs=2)
                            av_sb = avsbs[i]
                            # selects share the double-buffered chain-psum ring
                            sel_t = ppj.tile([128, 512], F32, tag="ps", bufs=2)
                            sel = sel_t[0:64, 0:qw]
                            nc.tensor.matmul(
                                sel, lhsT=ident[:, 64:128], rhs=av_sb[:, 0:qw],
                                start=True, stop=True,
                            )
                            if i == 0:
                                atf[0] = p3.tile([128, 512], BF16, tag="atf", bufs=2, name=f"atf{hp}_{q0}")
                                nc.vector.reciprocal_approx_fast(rec[:, 0:qw], sel)
                                nc.vector.tensor_mul(
                                    atf[0][0:64, 0:qw], av[i][0:64, 0:qw], rec[:, 0:qw]
                                )
                            else:
                                nc.vector.reciprocal_approx_fast(
                                    rec[:, 0:qw], av[i][0:64, 0:qw]
                                )
                                # out at base partition 64 with inputs at 0 is
                                # fine for TensorTensor (not for stt below)
                                nc.vector.tensor_mul(
                                    atf[0][64:128, 0:qw], sel, rec[:, 0:qw]
                                )

                    def at_split():
                        # per half so every AP in the stt shares its start
                        # partition (verifier requirement)
                        for i in range(2):
                            r0 = i * 64
                            hi = ATh[r0 : r0 + 64, hp, q0 : q0 + qw]
                            src_h = atf[0][r0 : r0 + 64, 0:qw]
                            nc.vector.tensor_copy(hi, src_h)
                            nc.vector.scalar_tensor_tensor(
                                ATl[r0 : r0 + 64, hp, q0 : q0 + qw],
                                src_h,
                                0.0,
                                hi,
                                ALU.add,
                                ALU.subtract,
                            )

                    blk["scores"] = scores
                    blk["av"] = emit_av
                    blk["realign_copy"] = realign_copy
                    blk["realign"] = realign
                    blk["at_split"] = at_split
                    return blk

                def outproj_chain(tt, half, wo_p):
                    c0 = half * 512
                    ps = ppj.tile([128, 512], F32, tag="ps", bufs=2)
                    tsl = slice(tt * 128, (tt + 1) * 128)
                    aa = (ATh, ATl)
                    ww = wo_p
                    k = 0
                    for tl, tr in ((0, 0), (1, 0), (0, 1)):
                        for j in range(EP):
                            nc.tensor.matmul(
                                ps,
                                lhsT=aa[tl][:, 2 * j : 2 * j + 2, tsl],
                                rhs=ww[tr][:, 2 * j : 2 * j + 2, :],
                                start=(k == 0),
                                stop=(k == 3 * EP - 1),
                                perf_mode=DR,
                            )
                            k += 1
                    osb = p3.tile([128, 512], BF16, tag="osb", bufs=2)
                    nc.vector.tensor_add(osb, ps, boB[:, c0 : c0 + 512])
                    nc.sync.dma_start(
                        out=out[tt * 128 : (tt + 1) * 128, c0 : c0 + 512], in_=osb
                    )

                # --- prologue. DMA device is serial, so emission order is
                # arrival order; order strictly by first need, with chains
                # emitted as soon as their data is in flight. Block 0 starts
                # after just k0 tb0/tb1, six v0 chains, and q0 tb0; the
                # remaining block-0 prerequisites stream in as its inner work.
                wp_k0 = alloc_panel(E)
                wp_q0 = alloc_panel(0)
                dma_panel(wp_k0, E, 0, 0, 128)
                dma_panel(wp_k0, E, 1, 0, 128)
                dma_x(0)
                dma_panel(wp_q0, 0, 0, 0, 128)
                dma_panel(wp_q0, 0, 1, 0, 128)
                wp_v0 = load_panel(2 * E)
                dma_x(1)
                dma_panel(wp_k0, E, 0, 128, 384)
                dma_panel(wp_k0, E, 1, 128, 384)
                dma_panel(wp_q0, 0, 0, 128, 384)
                dma_panel(wp_q0, 0, 1, 128, 384)
                dma_x(2)
                dma_x(3)
                kq_chain(wp_k0, "k", 0, 0, 0)
                kq_chain(wp_q0, "q", 0, 0, 0)

                pss = _att_es.enter_context(
                    tc.tile_pool(name="ps_s", bufs=2, space="PSUM")
                )
                pav = _att_es.enter_context(
                    tc.tile_pool(name="ps_av", bufs=2, space="PSUM")
                )
                # --- chain schedule. V panel 0 feeds block 0 just-in-time
                # (v_chain(kt) emitted at iteration kt); V panel 1 likewise
                # splits across blocks 7/8. K/Q coltile ct of panel p feeds
                # head pair hp = 4p+ct, consumed from block 2hp on — each
                # chain group is emitted in an earlier block.
                wp_v1 = [None]
                wp_k1 = [None]
                wp_q1 = [None]
                wo_panels = [None, None]

                def kq_group(kind, panel, ct):
                    wp = {("k", 0): wp_k0, ("q", 0): wp_q0}.get((kind, panel))
                    n = 4 if kind == "k" else 2
                    if wp is not None:
                        return [
                            (lambda t=tb: kq_chain(wp, kind, panel, ct, t))
                            for tb in range(n)
                        ]
                    box = wp_k1 if kind == "k" else wp_q1
                    return [
                        (lambda t=tb: kq_chain(box[0], kind, panel, ct, t))
                        for tb in range(n)
                    ]

                def spread(chs):
                    # spread chain closures over kt slots 3..14, clear of the
                    # deferred realign (kt1) and at-split (kt3) DVE bursts and
                    # their ppj-ring selects
                    n = len(chs)
                    slots = [12 + (i * 4) // n for i in range(n)]
                    inner = {}
                    for s, ch in zip(slots, chs):
                        inner.setdefault(s, []).append(ch)
                    return inner

                def v1_group(tts):
                    return [(lambda t=tt: v_chain(wp_v1[0], 1, t)) for tt in tts]

                # per-block inner chain assignments (block -> kt -> chains),
                # balanced so no block carries more than ~4 chains beyond the
                # attention work, with every group emitted before its
                # consumer block (K/Q ct of panel p -> block 2*(4p+ct);
                # V panel 1 -> block 8).
                k0g = {c: kq_group("k", 0, c) for c in (1, 2, 3)}
                q0g = {c: kq_group("q", 0, c) for c in (1, 2, 3)}
                k1g = {c: kq_group("k", 1, c) for c in (0, 1, 2, 3)}
                q1g = {c: kq_group("q", 1, c) for c in (0, 1, 2, 3)}
                b0_inner = {
                    tt + 1: [lambda t=tt: v_chain(wp_v0, 0, t)] for tt in range(15)
                }
                b0_inner[15].append(lambda: v_chain(wp_v0, 0, 15))
                b0_inner[3] = b0_inner.get(3, []) + [
                    lambda: kq_chain(wp_k0, "k", 0, 0, 1)
                ]
                b0_inner[6].append(lambda: kq_chain(wp_k0, "k", 0, 0, 2))
                b0_inner[10].append(lambda: kq_chain(wp_k0, "k", 0, 0, 3))
                b0_inner[12].append(lambda: kq_chain(wp_q0, "q", 0, 0, 1))
                binner = {
                    0: b0_inner,
                    1: spread(k0g[1] + q0g[1]),
                    2: spread(k0g[2]),
                    3: spread(q0g[2] + v1_group(range(0, 2))),
                    4: spread(k0g[3]),
                    5: spread(q0g[3] + v1_group(range(2, 4))),
                    6: spread(k1g[0] + q1g[0]),
                    7: spread(v1_group(range(4, 10))),
                    8: {tt: [ch] for tt, ch in zip(range(10, 16), v1_group(range(10, 16)))},
                    9: spread(k1g[1] + q1g[1][:1]),
                    10: spread(q1g[1][1:] + k1g[2]),
                    11: spread(q1g[2] + k1g[3][:2]),
                    12: spread(k1g[3][2:] + q1g[3][:1]),
                    13: spread(q1g[3][1:]),
                    15: {
                        8 + i: [
                            (lambda t=tt, h=half: outproj_chain(t, h, wo_panels[h]))
                        ]
                        for i, (tt, half) in enumerate(
                            (t, h) for t in range(4) for h in range(2)
                        )
                    },
                }

                blocks = [(hp, qb * 512, 512) for hp in range(HP) for qb in range(QB)]
                pending = None
                prev = None  # (blk, kt) still owing its AV
                for b, (hp, q0, qw) in enumerate(blocks):
                    if b == 1:
                        # 4-slot wp ring: v1 lands in a fresh slot
                        wp_v1[0] = load_panel(2 * E + 512)
                    if b == 6:
                        # k1/q1 reuse k0/q0 slots; every k0/q0 reader has
                        # been emitted by now, so the anti-deps resolve.
                        wp_k1[0] = load_panel(E + 512)
                        wp_q1[0] = load_panel(512)
                    if b == 12:
                        wo_panels[0] = load_wo_panel(0)
                        wo_panels[1] = load_wo_panel(512)
                    blk = make_block(hp, q0, qw)
                    inner = binner.get(b, {})
                    for kt in range(KT):
                        blk["scores"](kt)
                        if kt == 3 and pending is not None:
                            pending[1]()
                        if kt == 7 and pending is not None:
                            pending[2]()
                        for ch in inner.get(kt, ()):
                            ch()
                        if kt % 2 == 1:
                            if prev is not None:
                                prev[0]["av"](prev[1])
                                if prev[1] == KT // 2 - 1 and pending is not None:
                                    pending[0]()
                            prev = (blk, kt // 2)
                    pending = (blk["realign_copy"], blk["realign"], blk["at_split"])
                prev[0]["av"](prev[1])
                pending[0]()
                pending[1]()
                pending[2]()
                for tt in range(4, 8):
                    for half in range(2):
                        outproj_chain(tt, half, wo_panels[half])
                _att_es.close()


_NC = None


def _get_program():
    global _NC
    if _NC is None:
        _NC = build_program()
    return _NC


def _split8(a):
    hi = a.astype(E4M3)
    lo = (a - hi.astype(np.float32)).astype(E4M3)
    return hi, lo


def make_in_maps(x, Wqkv, bqkv, Wo, bo):
    Wqkv = np.asarray(Wqkv, np.float32)
    bqkv = np.asarray(bqkv, np.float32)
    Wo = np.asarray(Wo, np.float32)
    bo = np.asarray(bo, np.float32)
    x = np.asarray(x, np.float32)

    wqh = Wqkv.astype(E4M3)
    wql = (Wqkv - wqh.astype(np.float32)).astype(E5M2)
    woh = Wo.astype(E4M3)
    wol = (Wo - woh.astype(np.float32)).astype(E5M2)
    bqt = np.ascontiguousarray(bqkv[:E].reshape(ET, 128).T)  # [128, 8]
    bob = bqkv[2 * E :] @ Wo + bo  # folded V-bias + out-bias
    w = {
        "wqh": np.ascontiguousarray(wqh),
        "wql": np.ascontiguousarray(wql),
        "woh": np.ascontiguousarray(woh),
        "wol": np.ascontiguousarray(wol),
        "bqt": bqt.astype(np.float32),
        "bob": bob.astype(np.float32),
    }
    in_maps = []
    for c in range(N_CORES):
        b, s = divmod(c, 2)
        xb = x[b]
        if s == 1:
            xb = np.roll(xb, -NQ, axis=0)
        xT = np.ascontiguousarray(xb.T).reshape(ET, 128, SEQ)
        xh, xl = _split8(xT)
        in_maps.append(
            {"xth": np.ascontiguousarray(xh), "xtl": np.ascontiguousarray(xl), **w}
        )
    return in_maps


def gather_out(results):
    out = np.empty((4, SEQ, E), np.float32)
    for c in range(N_CORES):
        b, s = divmod(c, 2)
        out[b, s * NQ : (s + 1) * NQ] = results[c]["out"].astype(np.float32)
    return out


def kernel(x, Wqkv, bqkv, Wo, bo):
    from concourse.bass_utils import run_bass_kernel_spmd

    nc = _get_program()
    in_maps = make_in_maps(x, Wqkv, bqkv, Wo, bo)
    res = run_bass_kernel_spmd(nc, in_maps, core_ids=list(range(N_CORES)))
    return gather_out(res.results)
